# revision 1
# baseline (speedup 1.0000x reference)
"""PolarRnn (gated DPLR delta-rule linear RNN) Trainium2 Bass kernel.

Sharding: 8 cores = (batch b, sequence-half s). Each core processes 1024
tokens x full D=1024 (all 16 heads). The only cross-core dependency is the
recurrent state at the half boundary, passed via a pair AllGather.

Algorithm: chunk-parallel delta rule, chunk C=64. Per (head, chunk):
  g = cumsum(gk) (in-chunk), Lam = exp(g)
  Qh = q*Lam, Kh = k/Lam, Ah = a*Lam, Kbar = k*Lam_end/Lam
  W = strict_tril(Ah Kh^T);  M = tril(Qh Kh^T)
  (I-W)^{-1} via 3-term Neumann series (W entries are O(0.1): validated
   worst-case rel err 5.8e-6 on this problem's data)
  U_loc = Tinv V, Ma = Tinv Ah   (solved jointly, rhs [V | Ah])
  Pnd^ = Ma^T Kbar, GT = Qh^T + Ma^T M^T, dS = Kbar^T U_loc,
  OlocT = U_loc^T M^T
  scan: S' = LamEnd*S + Pnd S + dS ; out: O^T = S^T GT + OlocT
"""
import numpy as np

import concourse.bass as bass
import concourse.bacc as bacc
import concourse.mybir as mybir
import concourse.tile as tile
from concourse.masks import make_identity
from concourse.bass_utils import run_bass_kernel_spmd

F32 = mybir.dt.float32
F32R = mybir.dt.float32r
BF16 = mybir.dt.bfloat16

B, T, D, H, HD = 4, 2048, 1024, 16, 64
TT = 1024          # tokens per core
C = 64             # chunk
NCH = TT // C      # 16 chunks
KO = D // 128      # 8 k-tiles
NEU = 3            # Neumann order
AF = mybir.ActivationFunctionType
ALU = mybir.AluOpType


def r(ap):
    return ap.bitcast(F32R)


def build():
    nc = bacc.Bacc("TRN2", target_bir_lowering=False)
    xs = nc.dram_tensor("xs", [TT, D], F32, kind="ExternalInput")
    Wq = nc.dram_tensor("Wq", [D, D], F32, kind="ExternalInput")
    Wk = nc.dram_tensor("Wk", [D, D], F32, kind="ExternalInput")
    Wv = nc.dram_tensor("Wv", [D, D], F32, kind="ExternalInput")
    Wgamma = nc.dram_tensor("Wgamma", [D, H], F32, kind="ExternalInput")
    Wf1 = nc.dram_tensor("Wf1", [D, HD], F32, kind="ExternalInput")
    Wf2 = nc.dram_tensor("Wf2", [HD, D], F32, kind="ExternalInput")
    Wog1 = nc.dram_tensor("Wog1", [D, HD], F32, kind="ExternalInput")
    Wog2 = nc.dram_tensor("Wog2", [HD, D], F32, kind="ExternalInput")
    norm_w = nc.dram_tensor("norm_w", [D], F32, kind="ExternalInput")
    Wo = nc.dram_tensor("Wo", [D, D], F32, kind="ExternalInput")
    parity = nc.dram_tensor("parity", [1, 1], F32, kind="ExternalInput")
    import os
    DBG = os.environ.get("KDBG", "0") == "1"
    if DBG:
        dbg_gT = nc.dram_tensor("dbg_gT", [128, KO, TT], F32, kind="ExternalOutput")
        dbg_hatK = nc.dram_tensor("dbg_hatK", [128, KO, TT], BF16, kind="ExternalOutput")
        dbg_hatA = nc.dram_tensor("dbg_hatA", [128, KO, TT], BF16, kind="ExternalOutput")
        dbg_oT = nc.dram_tensor("dbg_oT", [128, KO, TT], F32, kind="ExternalOutput")
        dbg_va = nc.dram_tensor("dbg_va", [128, NCH // 2, H, 2 * HD], BF16, kind="ExternalOutput")
    else:
        dbg_gT = dbg_hatK = dbg_hatA = dbg_oT = dbg_va = None
    ys = nc.dram_tensor("ys", [TT, D], F32, kind="ExternalOutput")

    cc_in = nc.dram_tensor("cc_in", [H * HD, HD], F32)
    cc_out = nc.dram_tensor("cc_out", [2 * H * HD, HD], F32)
    d_va = nc.dram_tensor("d_va", [128, NCH // 2, H, 2 * HD], BF16)
    d_kbar = nc.dram_tensor("d_kbar", [128, NCH // 2, H, HD], BF16)
    d_ogT = nc.dram_tensor("d_ogT", [128, KO, TT], BF16)
    d_msk = nc.dram_tensor("d_msk", [64, 128], BF16)

    wq3 = Wq.rearrange("(ko p) f -> p ko f", p=128)
    wk3 = Wk.rearrange("(ko p) f -> p ko f", p=128)
    wv3 = Wv.rearrange("(ko p) f -> p ko f", p=128)
    wo3 = Wo.rearrange("(ko p) f -> p ko f", p=128)
    wg3 = Wgamma.rearrange("(ko p) f -> p ko f", p=128)
    wf13 = Wf1.rearrange("(ko p) f -> p ko f", p=128)
    wog13 = Wog1.rearrange("(ko p) f -> p ko f", p=128)
    x2 = xs.rearrange("(tt p) d -> p tt d", p=128)   # [128, 8, 1024]

    with tile.TileContext(nc) as tc:
        _body(nc, tc, locals())
    nc.compile()
    return nc


def _body(nc, tc, env):
    g = lambda n: env[n]
    xs, ys, cc_in, cc_out = g("xs"), g("ys"), g("cc_in"), g("cc_out")
    Wf2, Wog2, norm_w, parity = g("Wf2"), g("Wog2"), g("norm_w"), g("parity")
    wq3, wk3, wv3, wo3 = g("wq3"), g("wk3"), g("wv3"), g("wo3")
    wg3, wf13, wog13, x2 = g("wg3"), g("wf13"), g("wog13"), g("x2")
    d_va, d_kbar, d_ogT = g("d_va"), g("d_kbar"), g("d_ogT")
    d_msk = g("d_msk")
    DBG = g("DBG")
    dbg_gT, dbg_hatK, dbg_hatA = g("dbg_gT"), g("dbg_hatK"), g("dbg_hatA")
    dbg_oT, dbg_va = g("dbg_oT"), g("dbg_va")

    import contextlib
    ctx = contextlib.ExitStack()
    with ctx:
        ctx.enter_context(nc.allow_low_precision(
            reason="float32r operands rounded intentionally for PE rate"))
        g0 = ctx.enter_context(tc.tile_pool(name="g0", bufs=1))

        # ---- global constants / small state ----
        ident = g0.tile([128, 128], BF16, tag="ident")
        make_identity(nc, ident)
        ident32 = g0.tile([128, 128], F32, tag="ident32")
        make_identity(nc, ident32)
        cbuild = g0.tile([128, 4], F32, tag="cbuild")
        nc.vector.memset(cbuild, 0.0)
        nc.vector.memset(cbuild[:, 0:1], 1.0)
        nc.vector.memset(cbuild[0:64, 1:2], 1.0)
        nc.vector.memset(cbuild[64:128, 2:3], 1.0)
        ones128 = g0.tile([128, 1], F32R, tag="ones128")
        nc.scalar.copy(out=ones128, in_=cbuild[:, 0:1])
        ones_l2 = g0.tile([128, 2], F32R, tag="ones_l2")
        nc.scalar.copy(out=ones_l2[:, 0:1], in_=cbuild[:, 1:2])
        nc.scalar.copy(out=ones_l2[:, 1:2], in_=cbuild[:, 2:3])
        c2build = g0.tile([2, 128], F32, tag="c2build")
        nc.vector.memset(c2build, 1.0)
        # keep 1.0 where 0 <= y - 64*p < 64 (row p owns col block p)
        nc.gpsimd.affine_select(out=c2build, in_=c2build,
                                compare_op=ALU.is_ge, fill=0.0,
                                base=0, pattern=[[1, 128]],
                                channel_multiplier=-64)
        nc.gpsimd.affine_select(out=c2build, in_=c2build,
                                compare_op=ALU.is_ge, fill=0.0,
                                base=63, pattern=[[-1, 128]],
                                channel_multiplier=64)
        ones2T = g0.tile([2, 128], F32R, tag="ones2T")
        nc.scalar.copy(out=ones2T, in_=c2build)
        nc.vector.memset(c2build, 1.0)
        ones2F = g0.tile([2, 128], F32R, tag="ones2F")
        nc.scalar.copy(out=ones2F, in_=c2build)
        onesF = g0.tile([128, 2], F32R, tag="onesF")
        nc.scalar.copy(out=onesF[:, 0:1], in_=cbuild[:, 0:1])
        nc.scalar.copy(out=onesF[:, 1:2], in_=cbuild[:, 0:1])
        maskWM = g0.tile([128, 128], BF16, tag="maskWM")
        nc.vector.memset(maskWM, 1.0)
        # build [0:64] half at base 0 (unambiguous), mirror to [64:128] via DMA
        nc.gpsimd.affine_select(
            out=maskWM[0:64, 0:64], in_=maskWM[0:64, 0:64],
            compare_op=ALU.is_ge, fill=0.0,
            base=-1, pattern=[[1, 64]], channel_multiplier=-1)
        nc.gpsimd.affine_select(
            out=maskWM[0:64, 64:128], in_=maskWM[0:64, 64:128],
            compare_op=ALU.is_ge, fill=0.0,
            base=0, pattern=[[1, 64]], channel_multiplier=-1)
        nc.sync.dma_start(out=d_msk[:, :], in_=maskWM[0:64, :])
        nc.sync.dma_start(out=maskWM[64:128, :], in_=d_msk[:, :])
        nw_sb = g0.tile([128, KO], F32, tag="nw_sb")
        nc.sync.dma_start(out=nw_sb, in_=norm_w.rearrange("(ko p) -> p ko", p=128))
        par_sb = g0.tile([1, 1], F32R, tag="par_sb")
        nc.sync.dma_start(out=par_sb, in_=parity[:, :].bitcast(F32R))
        eps_sb = g0.tile([2, 1], F32, tag="eps_sb")
        nc.vector.memset(eps_sb, 1e-5)
        lamEnd = g0.tile([128, KO, NCH], F32, tag="lamEnd")

        # hats span P1 (production) and P2 (phase A)
        spanA = ctx.enter_context(tc.tile_pool(name="spanA", bufs=1))
        hatK = spanA.tile([128, KO, TT], BF16, tag="hatK")
        hatA = spanA.tile([128, KO, TT], BF16, tag="hatA")
        hatQ = spanA.tile([128, KO, TT], BF16, tag="hatQ")

        # ============ P1: projections + hat tensors ============
        with tc.tile_pool(name="p1", bufs=1) as p1, \
             tc.tile_pool(name="wpool", bufs=1) as wpool, \
             tc.tile_pool(name="tmp", bufs=2) as tmp, \
             tc.tile_pool(name="ktmp", bufs=2) as ktmp_pool, \
             tc.tile_pool(name="small", bufs=1) as small, \
             tc.tile_pool(name="ps1", bufs=3, space="PSUM") as pswide, \
             tc.tile_pool(name="pst1", bufs=2, space="PSUM") as pstp, \
             tc.tile_pool(name="pn1", bufs=1, space="PSUM") as ppool, \
             tc.tile_pool(name="bc1", bufs=2, space="PSUM") as bcpool:
            xT = p1.tile([128, KO, TT], F32R, tag="xT")
            gT = p1.tile([128, KO, TT], F32, tag="gT")
            gamT = p1.tile([16, TT], F32, tag="gamT")
            f1T = p1.tile([64, TT], F32R, tag="f1T")
            og1T = p1.tile([64, TT], F32R, tag="og1T")
            wgam_sb = p1.tile([128, KO, H], F32R, tag="wgam_sb")
            nc.sync.dma_start(out=wgam_sb, in_=wg3.bitcast(F32R))
            wf1_sb = p1.tile([128, KO, HD], F32R, tag="wf1_sb")
            nc.sync.dma_start(out=wf1_sb, in_=wf13.bitcast(F32R))
            wog1_sb = p1.tile([128, KO, HD], F32R, tag="wog1_sb")
            nc.sync.dma_start(out=wog1_sb, in_=wog13.bitcast(F32R))
            wf2_sb = p1.tile([64, D], F32R, tag="wf2_sb")
            nc.sync.dma_start(out=wf2_sb, in_=Wf2[:, :].bitcast(F32R))
            wog2_sb = p1.tile([64, D], F32R, tag="wog2_sb")
            nc.sync.dma_start(out=wog2_sb, in_=Wog2[:, :].bitcast(F32R))

            # x -> xT (PE transpose)
            for tt in range(KO):
                xrow = tmp.tile([128, D], F32, tag="big1")
                nc.sync.dma_start(out=xrow, in_=x2[:, tt, :])
                for j in range(KO):
                    pst = pstp.tile([128, 128], F32, tag="pst")
                    nc.tensor.transpose(pst, xrow[:, j * 128:(j + 1) * 128],
                                        ident32)
                    nc.scalar.copy(out=xT[:, j, tt * 128:(tt + 1) * 128],
                                   in_=pst)

            def proj_T(wap, dout, evac):
                for do0 in range(0, dout, 128):
                    dp = min(128, dout - do0)
                    for tb in range(2):
                        ps = pswide.tile([128, 512], F32, tag="projT")
                        for ko in range(KO):
                            nc.tensor.matmul(
                                ps[:dp, :], r(wap[:, ko, do0:do0 + dp]),
                                r(xT[:, ko, tb * 512:(tb + 1) * 512]),
                                start=(ko == 0), stop=(ko == KO - 1))
                        evac(ps, do0, tb)

            def ev_gam(ps, do0, tb):
                nc.scalar.activation(out=gamT[:, tb * 512:(tb + 1) * 512],
                                     in_=ps[:16, :], func=AF.Sigmoid)
            proj_T(wgam_sb, 16, ev_gam)

            def ev_f1(ps, do0, tb):
                nc.scalar.copy(out=f1T[:, tb * 512:(tb + 1) * 512],
                               in_=ps[:64, :])
            proj_T(wf1_sb, HD, ev_f1)

            def ev_og1(ps, do0, tb):
                nc.scalar.copy(out=og1T[:, tb * 512:(tb + 1) * 512],
                               in_=ps[:64, :])
            proj_T(wog1_sb, HD, ev_og1)

            # og gate -> sigmoid -> DRAM
            for do0 in range(0, D, 128):
                for tb in range(2):
                    ps = pswide.tile([128, 512], F32, tag="projT")
                    nc.tensor.matmul(ps, r(wog2_sb[:, do0:do0 + 128]),
                                     r(og1T[:, tb * 512:(tb + 1) * 512]),
                                     start=True, stop=True)
                    ogt = tmp.tile([128, 512], BF16, tag="ogt")
                    nc.scalar.activation(out=ogt, in_=ps, func=AF.Sigmoid)
                    nc.sync.dma_start(
                        out=d_ogT[:, do0 // 128, tb * 512:(tb + 1) * 512],
                        in_=ogt)

            # gk -> cumsum gT, lamEnd
            for do0 in range(0, D, 128):
                ko = do0 // 128
                for tb in range(2):
                    ps = pswide.tile([128, 512], F32, tag="projT")
                    nc.tensor.matmul(ps, r(wf2_sb[:, do0:do0 + 128]),
                                     r(f1T[:, tb * 512:(tb + 1) * 512]),
                                     start=True, stop=True)
                    sp = tmp.tile([128, 512], F32, tag="big1")
                    nc.scalar.activation(out=sp, in_=ps, func=AF.Sigmoid)
                    nc.scalar.activation(out=sp, in_=sp, func=AF.Ln)
                    for cc in range(8):
                        c = tb * 8 + cc
                        nc.vector.tensor_tensor_scan(
                            out=gT[:, ko, c * C:(c + 1) * C],
                            data0=sp[:, cc * C:(cc + 1) * C],
                            data1=sp[:, cc * C:(cc + 1) * C],
                            initial=0.0, op0=ALU.add, op1=ALU.bypass)
                nc.scalar.activation(out=lamEnd[:, ko, :],
                                     in_=gT[:, ko, C - 1::C], func=AF.Exp)

            # gamma in t-layout (negated): gam_t[tok, tt, h] = -sigmoid(...)
            gam_t = spanA.tile([128, KO, 16], F32, tag="gam_t")
            for tt in range(KO):
                pst = pstp.tile([128, 128], F32, tag="pst")
                nc.tensor.transpose(
                    pst[:, 0:16], gamT[:, tt * 128:(tt + 1) * 128],
                    ident32[0:16, 0:16])
                nc.vector.tensor_scalar_mul(gam_t[:, tt, :], pst[:, 0:16],
                                            -1.0)

            # v (t-layout, straight orientation) -> DRAM d_va
            wslot = wpool.tile([128, KO, D], F32R, tag="wslot")
            nc.sync.dma_start(out=wslot, in_=wv3.bitcast(F32R))
            for tt in range(KO):
                for nb in range(2):
                    ps = pswide.tile([128, 512], F32, tag="projT")
                    for ko in range(KO):
                        nc.tensor.matmul(
                            ps, r(xT[:, ko, tt * 128:(tt + 1) * 128]),
                            r(wslot[:, ko, nb * 512:(nb + 1) * 512]),
                            start=(ko == 0), stop=(ko == KO - 1))
                    vb = tmp.tile([128, 512], BF16, tag="ogt")
                    nc.vector.tensor_copy(out=vb, in_=ps)
                    nc.sync.dma_start(
                        out=d_va[:, tt, nb * 8:(nb + 1) * 8, 0:HD],
                        in_=vb.rearrange("p (h d) -> p h d", d=HD))

            # q -> hatQ
            wslot = wpool.tile([128, KO, D], F32R, tag="wslot")
            nc.sync.dma_start(out=wslot, in_=wq3.bitcast(F32R))
            for do0 in range(0, D, 128):
                ko = do0 // 128
                for tb in range(2):
                    ps = pswide.tile([128, 512], F32, tag="projT")
                    for kk in range(KO):
                        nc.tensor.matmul(
                            ps, r(wslot[:, kk, do0:do0 + 128]),
                            r(xT[:, kk, tb * 512:(tb + 1) * 512]),
                            start=(kk == 0), stop=(kk == KO - 1))
                    qs = tmp.tile([128, 512], F32, tag="big1")
                    nc.scalar.activation(out=qs, in_=ps, func=AF.Silu)
                    eg = tmp.tile([128, 512], F32, tag="big2")
                    nc.scalar.activation(out=eg,
                                         in_=gT[:, ko, tb * 512:(tb + 1) * 512],
                                         func=AF.Exp)
                    nc.vector.tensor_mul(
                        out=hatQ[:, ko, tb * 512:(tb + 1) * 512],
                        in0=qs, in1=eg)

            # k -> hatK, hatA (resident) + kbar (-> DRAM), A_t (-> DRAM)
            wslot = wpool.tile([128, KO, D], F32R, tag="wslot")
            nc.sync.dma_start(out=wslot, in_=wk3.bitcast(F32R))
            for do0 in range(0, D, 128):
                ko = do0 // 128
                for tb in range(2):
                    ps = pswide.tile([128, 512], F32, tag="projT")
                    for kk in range(KO):
                        nc.tensor.matmul(
                            ps, r(wslot[:, kk, do0:do0 + 128]),
                            r(xT[:, kk, tb * 512:(tb + 1) * 512]),
                            start=(kk == 0), stop=(kk == KO - 1))
                    ks = ktmp_pool.tile([128, 512], F32, tag="ks")
                    nc.scalar.activation(out=ks, in_=ps, func=AF.Silu)
                    k2 = tmp.tile([128, 512], F32R, tag="k2r")
                    nc.vector.tensor_mul(out=k2, in0=ks, in1=ks)
                    pn = ppool.tile([2, 512], F32, tag="pn")
                    nc.tensor.matmul(pn, r(ones_l2), r(k2),
                                     start=True, stop=True)
                    nrm = small.tile([2, 512], F32R, tag="nrm")
                    nc.vector.tensor_scalar_max(nrm, pn, 1e-24)
                    nc.scalar.activation(out=nrm, in_=nrm, func=AF.Sqrt)
                    nc.vector.reciprocal(out=nrm, in_=nrm)
                    bcn = bcpool.tile([128, 512], F32, tag="bc")
                    nc.tensor.matmul(bcn, r(ones2T), r(nrm),
                                     start=True, stop=True)
                    nc.vector.tensor_mul(out=ks, in0=ks, in1=bcn)
                    egn = tmp.tile([128, 512], F32, tag="big2")
                    nc.scalar.activation(
                        out=egn, in_=gT[:, ko, tb * 512:(tb + 1) * 512],
                        func=AF.Exp, scale=-1.0)
                    nc.vector.tensor_mul(
                        out=hatK[:, ko, tb * 512:(tb + 1) * 512],
                        in0=ks, in1=egn)
                    # hatA = kn * (-gamma) * exp(2g - g_prev)
                    twog = tmp.tile([128, 512], F32, tag="big3")
                    gsl = gT[:, ko, tb * 512:(tb + 1) * 512]
                    nc.vector.tensor_scalar_mul(twog, gsl, 2.0)
                    for cc in range(8):
                        sl = slice(cc * C + 1, (cc + 1) * C)
                        slp = slice(cc * C, (cc + 1) * C - 1)
                        nc.vector.tensor_sub(out=twog[:, sl], in0=twog[:, sl],
                                             in1=gsl[:, slp])
                    ea = tmp.tile([128, 512], F32, tag="big1")
                    nc.scalar.activation(out=ea, in_=twog, func=AF.Exp)
                    nc.vector.tensor_mul(out=ea, in0=ea, in1=ks)
                    nc.vector.tensor_copy(
                        out=hatA[:, ko, tb * 512:(tb + 1) * 512], in_=ea)
                    # kbar = kn * lamEnd * exp(-g) ; transpose -> DRAM
                    kb = tmp.tile([128, 512], F32, tag="big4")
                    nc.vector.tensor_mul(out=kb, in0=ks, in1=egn)
                    kbarT = ktmp_pool.tile([128, 512], BF16, tag="kbarT")
                    for cc in range(8):
                        c = tb * 8 + cc
                        nc.vector.tensor_scalar_mul(
                            kbarT[:, cc * C:(cc + 1) * C],
                            kb[:, cc * C:(cc + 1) * C],
                            lamEnd[:, ko, c:c + 1])
                    for cp in range(4):
                        c2 = tb * 8 + cp * 2
                        pst = pstp.tile([128, 128], BF16, tag="pst")
                        nc.tensor.transpose(
                            pst, kbarT[:, cp * 128:(cp + 1) * 128], ident)
                        kt = small.tile([128, 128], BF16, tag="kt")
                        nc.vector.tensor_copy(out=kt, in_=pst)
                        nc.sync.dma_start(
                            out=d_kbar[:, c2 // 2, 2 * ko:2 * ko + 2, :],
                            in_=kt.rearrange("p (h d) -> p h d", d=HD))

            # A_t transposes -> DRAM d_va[...,HD:2HD]
            for ko in range(KO):
                for cp in range(NCH // 2):
                    pst = pstp.tile([128, 128], BF16, tag="pst")
                    nc.tensor.transpose(
                        pst, hatA[:, ko, cp * 128:(cp + 1) * 128], ident)
                    at = small.tile([128, 128], BF16, tag="kt")
                    nc.vector.tensor_scalar_mul(
                        at[:, 0:HD], pst[:, 0:HD],
                        gam_t[:, cp, 2 * ko:2 * ko + 1])
                    nc.vector.tensor_scalar_mul(
                        at[:, HD:2 * HD], pst[:, HD:2 * HD],
                        gam_t[:, cp, 2 * ko + 1:2 * ko + 2])
                    nc.sync.dma_start(
                        out=d_va[:, cp, 2 * ko:2 * ko + 2, HD:2 * HD],
                        in_=at.rearrange("p (h d) -> p h d", d=HD))

            if DBG:
                nc.sync.dma_start(out=dbg_gT[:, :, :], in_=gT)

        # ============ P2: phase A (per head, chunk) ============
        spanB = ctx.enter_context(tc.tile_pool(name="spanB", bufs=1))
        pnd_all = spanB.tile([128, KO, NCH, HD], BF16, tag="pnd_all")
        ds_all = spanB.tile([128, KO, NCH, HD], BF16, tag="ds_all")
        gt_all = spanB.tile([128, KO, NCH, HD], BF16, tag="gt_all")
        oloc_all = spanB.tile([128, KO, NCH, HD], BF16, tag="oloc_all")

        with tc.tile_pool(name="p2", bufs=1) as p2, \
             tc.tile_pool(name="wm", bufs=6) as wm_pool, \
             tc.tile_pool(name="xp", bufs=6) as x_pool, \
             tc.tile_pool(name="psA", bufs=6, space="PSUM") as psA:
            va = p2.tile([128, NCH // 2, H, 2 * HD], BF16, tag="va")
            nc.sync.dma_start(out=va, in_=d_va[:, :, :, :])
            kbar_t = p2.tile([128, NCH // 2, H, HD], BF16, tag="kbar_t")
            nc.sync.dma_start(out=kbar_t, in_=d_kbar[:, :, :, :])

            if DBG:
                nc.sync.dma_start(out=dbg_va[:, :, :, :], in_=va)
                nc.sync.dma_start(out=dbg_hatK[:, :, :], in_=hatK)
                nc.sync.dma_start(out=dbg_hatA[:, :, :], in_=hatA)
            for c in range(NCH):
                cb = (c % 2) * 64
                for h in range(H):
                    hb = (h % 2) * 64
                    ko = h // 2
                    kslT = hatK[hb:hb + 64, ko, c * C:(c + 1) * C]
                    asl = hatA[hb:hb + 64, ko, c * C:(c + 1) * C]
                    qsl = hatQ[hb:hb + 64, ko, c * C:(c + 1) * C]
                    va_u = va[cb:cb + 64, c // 2, h, :]
                    kb_u = kbar_t[cb:cb + 64, c // 2, h, :]
                    psg = psA.tile([128, 128], F32, tag="psA")
                    nc.tensor.matmul(psg[cb:cb + 64, 0:64], kslT, asl,
                                     start=True, stop=True,
                                     tile_position=(hb, cb))
                    nc.tensor.matmul(psg[cb:cb + 64, 64:128], kslT, qsl,
                                     start=True, stop=True,
                                     tile_position=(hb, cb))
                    wm = wm_pool.tile([128, 128], BF16, tag="wm")
                    nc.vector.tensor_mul(out=wm[cb:cb + 64, :],
                                         in0=psg[cb:cb + 64, :],
                                         in1=maskWM[cb:cb + 64, :])
                    gcol = gam_t[cb:cb + 64, c // 2, h:h + 1]
                    xcur = va_u
                    for it in range(NEU):
                        psx = psA.tile([128, 128], F32, tag="psA")
                        nc.tensor.matmul(psx[cb:cb + 64, :],
                                         wm[cb:cb + 64, 0:64], xcur,
                                         start=True, stop=True,
                                         tile_position=(cb, cb))
                        xn = x_pool.tile([128, 128], BF16, tag="xn")
                        nc.vector.tensor_scalar_mul(xn[cb:cb + 64, :],
                                                    psx[cb:cb + 64, :], gcol)
                        nc.vector.tensor_add(out=xn[cb:cb + 64, :],
                                             in0=xn[cb:cb + 64, :], in1=va_u)
                        xcur = xn[cb:cb + 64, :]
                    um = xcur
                    psp = psA.tile([128, 128], F32, tag="psA")
                    nc.tensor.matmul(psp[hb:hb + 64, 0:64], um[:, 64:128],
                                     kb_u, start=True, stop=True,
                                     tile_position=(cb, hb))
                    nc.tensor.matmul(psp[hb:hb + 64, 64:128], um[:, 64:128],
                                     wm[cb:cb + 64, 64:128],
                                     start=True, stop=True,
                                     tile_position=(cb, hb))
                    nc.vector.tensor_copy(out=pnd_all[hb:hb + 64, ko, c, :],
                                          in_=psp[hb:hb + 64, 0:64])
                    nc.vector.tensor_add(
                        out=gt_all[hb:hb + 64, ko, c, :],
                        in0=psp[hb:hb + 64, 64:128],
                        in1=hatQ[hb:hb + 64, ko, c * C:(c + 1) * C])
                    psd = psA.tile([128, 128], F32, tag="psA")
                    nc.tensor.matmul(psd[hb:hb + 64, 0:64], kb_u,
                                     um[:, 0:64], start=True, stop=True,
                                     tile_position=(cb, hb))
                    nc.tensor.matmul(psd[hb:hb + 64, 64:128], um[:, 0:64],
                                     wm[cb:cb + 64, 64:128],
                                     start=True, stop=True,
                                     tile_position=(cb, hb))
                    nc.vector.tensor_copy(out=ds_all[hb:hb + 64, ko, c, :],
                                          in_=psd[hb:hb + 64, 0:64])
                    nc.vector.tensor_copy(out=oloc_all[hb:hb + 64, ko, c, :],
                                          in_=psd[hb:hb + 64, 64:128])

        # ============ P3: scans, outputs, layernorm, Wo ============
        with tc.tile_pool(name="p3", bufs=1) as p3, \
             tc.tile_pool(name="wpool3", bufs=1) as wpool3, \
             tc.tile_pool(name="tmp3", bufs=2) as tmp3, \
             tc.tile_pool(name="psS", bufs=2, space="PSUM") as psS, \
             tc.tile_pool(name="ln3", bufs=3, space="PSUM") as ppool3, \
             tc.tile_pool(name="ps3", bufs=3, space="PSUM") as pswide3:
            scur = p3.tile([128, KO, HD], F32, tag="scur")
            sbf = p3.tile([128, KO, HD], BF16, tag="sbf")
            sinit = p3.tile([128, KO, HD], F32, tag="sinit")
            stat_mu = p3.tile([2, TT], F32R, tag="stat_mu")
            stat_rs = p3.tile([2, TT], F32R, tag="stat_rs")
            nc.vector.memset(scur, 0.0)

            def scan_step(c, h):
                hb = (h % 2) * 64
                ko = h // 2
                pss = psS.tile([128, HD], F32, tag="pss")
                nc.tensor.matmul(pss[hb:hb + 64, :],
                                 pnd_all[hb:hb + 64, ko, c, :],
                                 sbf[hb:hb + 64, ko, :],
                                 start=True, stop=True,
                                 tile_position=(hb, hb))
                nc.vector.tensor_scalar_mul(scur[hb:hb + 64, ko, :],
                                            scur[hb:hb + 64, ko, :],
                                            lamEnd[hb:hb + 64, ko, c:c + 1])
                nc.vector.tensor_add(out=scur[hb:hb + 64, ko, :],
                                     in0=scur[hb:hb + 64, ko, :],
                                     in1=pss[hb:hb + 64, :])
                nc.vector.tensor_add(out=scur[hb:hb + 64, ko, :],
                                     in0=scur[hb:hb + 64, ko, :],
                                     in1=ds_all[hb:hb + 64, ko, c, :])

            for c in range(NCH):
                for h in range(H):
                    hb = (h % 2) * 64
                    ko = h // 2
                    nc.vector.tensor_copy(out=sbf[hb:hb + 64, ko, :],
                                          in_=scur[hb:hb + 64, ko, :])
                    scan_step(c, h)

            cin3 = cc_in.rearrange("(ko p) f -> p ko f", p=128)
            cout3 = cc_out.rearrange("(r ko p) f -> r p ko f", p=128, r=2)
            nc.sync.dma_start(out=cin3, in_=scur)
            nc.gpsimd.collective_compute(
                "AllGather", ALU.bypass,
                replica_groups=[[0, 1], [2, 3], [4, 5], [6, 7]],
                ins=[cc_in[:, :]], outs=[cc_out[:, :]])
            nc.sync.dma_start(out=sinit, in_=cout3[0])
            par_col = p3.tile([128, 1], F32, tag="par_col")
            nc.sync.dma_start(out=par_col,
                              in_=parity[0:1, 0:1].to_broadcast((128, 1)))
            nc.vector.tensor_scalar_mul(scur, sinit, par_col)

            oT = p3.tile([128, KO, TT], F32R, tag="oT")
            for c in range(NCH):
                for h in range(H):
                    hb = (h % 2) * 64
                    ko = h // 2
                    nc.vector.tensor_copy(out=sbf[hb:hb + 64, ko, :],
                                          in_=scur[hb:hb + 64, ko, :])
                    pso = psS.tile([128, HD], F32, tag="pss")
                    nc.tensor.matmul(pso[hb:hb + 64, :],
                                     sbf[hb:hb + 64, ko, :],
                                     gt_all[hb:hb + 64, ko, c, :],
                                     start=True, stop=True,
                                     tile_position=(hb, hb))
                    osl = oT[hb:hb + 64, ko, c * C:(c + 1) * C]
                    nc.vector.tensor_add(
                        out=osl, in0=pso[hb:hb + 64, :],
                        in1=oloc_all[hb:hb + 64, ko, c, :])
                    scan_step(c, h)

            if DBG:
                nc.sync.dma_start(out=dbg_oT[:, :, :], in_=oT.bitcast(F32))
            # gate + layernorm stats
            for ko in range(KO):
                for tb in range(2):
                    ogt = tmp3.tile([128, 512], BF16, tag="ogt3")
                    nc.sync.dma_start(
                        out=ogt, in_=d_ogT[:, ko, tb * 512:(tb + 1) * 512])
                    nc.vector.tensor_mul(
                        out=oT[:, ko, tb * 512:(tb + 1) * 512],
                        in0=oT[:, ko, tb * 512:(tb + 1) * 512], in1=ogt)
            for tb in range(2):
                psm = ppool3.tile([2, 512], F32, tag="acc")
                for ko in range(KO):
                    nc.tensor.matmul(psm, r(onesF),
                                     r(oT[:, ko, tb * 512:(tb + 1) * 512]),
                                     start=(ko == 0), stop=(ko == KO - 1))
                # both rows hold the full-D sum; fold the K=2 bcast double
                nc.vector.tensor_scalar_mul(
                    stat_mu[:, tb * 512:(tb + 1) * 512], psm, 0.5 / D)
                ps2 = ppool3.tile([2, 512], F32, tag="acc")
                for ko in range(KO):
                    o2 = tmp3.tile([128, 512], F32R, tag="o2")
                    nc.vector.tensor_mul(
                        out=o2, in0=oT[:, ko, tb * 512:(tb + 1) * 512],
                        in1=oT[:, ko, tb * 512:(tb + 1) * 512])
                    nc.tensor.matmul(ps2, r(onesF), r(o2),
                                     start=(ko == 0), stop=(ko == KO - 1))
                msq = ppool3.tile([2, 512], F32, tag="acc")
                nc.vector.tensor_scalar_mul(msq, ps2, 1.0 / D)
                mu2 = p3.tile([2, 512], F32, tag="mu2")
                # stat_mu holds mu/2: mu^2 = 4 * (mu/2)^2
                nc.vector.tensor_mul(out=mu2,
                                     in0=stat_mu[:, tb * 512:(tb + 1) * 512],
                                     in1=stat_mu[:, tb * 512:(tb + 1) * 512])
                nc.vector.tensor_scalar_mul(mu2, mu2, 4.0)
                var = p3.tile([2, 512], F32, tag="var")
                nc.vector.tensor_sub(out=var, in0=msq, in1=mu2)
                nc.scalar.activation(out=var, in_=var, func=AF.Sqrt,
                                     bias=eps_sb)
                nc.vector.reciprocal(out=var, in_=var)
                nc.vector.tensor_scalar_mul(
                    stat_rs[:, tb * 512:(tb + 1) * 512], var, 0.5)
            for tb in range(2):
                bmu = pswide3.tile([128, 512], F32, tag="projT")
                nc.tensor.matmul(bmu, r(ones2F),
                                 r(stat_mu[:, tb * 512:(tb + 1) * 512]),
                                 start=True, stop=True)
                brs = pswide3.tile([128, 512], F32, tag="projT")
                nc.tensor.matmul(brs, r(ones2F),
                                 r(stat_rs[:, tb * 512:(tb + 1) * 512]),
                                 start=True, stop=True)
                for ko in range(KO):
                    osl = oT[:, ko, tb * 512:(tb + 1) * 512]
                    nc.vector.tensor_sub(out=osl, in0=osl, in1=bmu)
                    nc.vector.tensor_mul(out=osl, in0=osl, in1=brs)
                    nc.vector.tensor_scalar_mul(osl, osl, nw_sb[:, ko:ko + 1])

            # final Wo
            wo_sb = wpool3.tile([128, KO, D], F32R, tag="wslot3")
            nc.sync.dma_start(out=wo_sb, in_=wo3.bitcast(F32R))
            y2 = ys.rearrange("(tt p) d -> p tt d", p=128)
            for tt in range(KO):
                yrow = tmp3.tile([128, D], F32, tag="o2")
                for nb in range(2):
                    ps = pswide3.tile([128, 512], F32, tag="projT")
                    for ko in range(KO):
                        nc.tensor.matmul(
                            ps, r(oT[:, ko, tt * 128:(tt + 1) * 128]),
                            r(wo_sb[:, ko, nb * 512:(nb + 1) * 512]),
                            start=(ko == 0), stop=(ko == KO - 1))
                    nc.scalar.copy(out=yrow[:, nb * 512:(nb + 1) * 512],
                                   in_=ps)
                nc.sync.dma_start(out=y2[:, tt, :], in_=yrow)


_NC = None


def _get_nc():
    global _NC
    if _NC is None:
        _NC = build()
    return _NC


def kernel(**inputs):
    nc = _get_nc()
    x = np.ascontiguousarray(np.asarray(inputs["x"], dtype=np.float32))
    names = ["Wq", "Wk", "Wv", "Wgamma", "Wf1", "Wf2", "Wog1", "Wog2",
             "norm_w", "Wo"]
    w = {n: np.ascontiguousarray(np.asarray(inputs[n], np.float32))
         for n in names}
    in_maps = []
    for core in range(8):
        b, half = core // 2, core % 2
        m = dict(w)
        m["xs"] = np.ascontiguousarray(x[b, half * TT:(half + 1) * TT, :])
        m["parity"] = np.array([[float(half)]], np.float32)
        in_maps.append(m)
    res = run_bass_kernel_spmd(nc, in_maps, core_ids=list(range(8)))
    out = np.empty((B, T, D), np.float32)
    for core in range(8):
        b, half = core // 2, core % 2
        out[b, half * TT:(half + 1) * TT, :] = res.results[core]["ys"]
    return out



# revision 24
# speedup vs baseline: 1.8860x; 1.8860x over previous
"""PolarRnn (gated DPLR delta-rule linear RNN) Trainium2 Bass kernel.

Sharding: 8 cores = (batch b, sequence-half s). Each core processes 1024
tokens x full D=1024 (all 16 heads). Cross-core dependency: recurrent state
at the half boundary via a pair AllGather + parity-masked second scan pass.

Algorithm: chunk-parallel delta rule, chunk C=128 (one partition block).
Numerics: decay exponents are kept 64-block-local and mid-shifted (range
e^{+-23}) so all stored "hat" factors and all PE products stay finite in
bf16/fp32; cross-block couplings are restored with per-partition exp columns
(crossM/kshift/qshift).

Per (head h, chunk c), blocks b1/b2, token gamma < 0:
  W^T/M^T via 8 quadrant matmuls of hatK/hatA/hatQ (+ scaled K1c for the
  b1->b2 coupling); wm = (psg * gamma_s) * mask  [one fused STT]
  z-substitution: solve z = va' + (W Gamma) z by NEU Neumann iterations,
  va' = [V/gamma | A_true^T], so no per-iteration gamma scaling is needed.
  pnd = z_a^T Kbar', gt = z_a^T (Gamma M^T), oloc = z_u^T (Gamma M^T),
  dS = Kbar'^T z_u, with Kbar' = Gamma Kbar.
Scan (per chunk, all 16 heads batched): S' = lamEnd*S + pnd_bd^T S + dS;
outputs O^T = oloc + S^T(gt + Qglob^T) via block-diag sbf matmuls.
"""
import numpy as np

import concourse.bass as bass
import concourse.bacc as bacc
import concourse.mybir as mybir
import concourse.tile as tile
from concourse.masks import make_identity
from concourse.bass_utils import run_bass_kernel_spmd

F32 = mybir.dt.float32
F32R = mybir.dt.float32r
BF16 = mybir.dt.bfloat16

B, T, D, H, HD = 4, 2048, 1024, 16, 64
TT = 1024          # tokens per core
C = 128            # chunk (= partition block)
NCH = TT // C      # 8 chunks
KO = D // 128      # 8 feature blocks (2 heads each)
NEU = 3            # Neumann iterations
AF = mybir.ActivationFunctionType
ALU = mybir.AluOpType


def r(ap):
    return ap.bitcast(F32R)


def build():
    nc = bacc.Bacc("TRN2", target_bir_lowering=False)
    xs = nc.dram_tensor("xs", [TT, D], F32, kind="ExternalInput")
    Wq = nc.dram_tensor("Wq", [D, D], F32, kind="ExternalInput")
    Wk = nc.dram_tensor("Wk", [D, D], F32, kind="ExternalInput")
    Wv = nc.dram_tensor("Wv", [D, D], F32, kind="ExternalInput")
    Wgamma = nc.dram_tensor("Wgamma", [D, H], F32, kind="ExternalInput")
    Wf1 = nc.dram_tensor("Wf1", [D, HD], F32, kind="ExternalInput")
    Wf2 = nc.dram_tensor("Wf2", [HD, D], F32, kind="ExternalInput")
    Wog1 = nc.dram_tensor("Wog1", [D, HD], F32, kind="ExternalInput")
    Wog2 = nc.dram_tensor("Wog2", [HD, D], F32, kind="ExternalInput")
    norm_w = nc.dram_tensor("norm_w", [D], F32, kind="ExternalInput")
    Wo = nc.dram_tensor("Wo", [D, D], F32, kind="ExternalInput")
    parity = nc.dram_tensor("parity", [1, 1], F32, kind="ExternalInput")
    import os
    DBG = os.environ.get("KDBG", "0") == "1"
    if DBG:
        dbg_gT = nc.dram_tensor("dbg_gT", [128, KO, TT], F32, kind="ExternalOutput")
        dbg_hatK = nc.dram_tensor("dbg_hatK", [128, KO, TT], BF16, kind="ExternalOutput")
        dbg_hatA = nc.dram_tensor("dbg_hatA", [128, KO, TT], BF16, kind="ExternalOutput")
        dbg_hatQ = nc.dram_tensor("dbg_hatQ", [128, KO, TT], BF16, kind="ExternalOutput")
        dbg_va = nc.dram_tensor("dbg_va", [128, NCH, H, C], BF16, kind="ExternalOutput")
        dbg_kb = nc.dram_tensor("dbg_kb", [128, NCH, H, HD], BF16, kind="ExternalOutput")
        dbg_pnd = nc.dram_tensor("dbg_pnd", [128, KO, NCH, 128], BF16, kind="ExternalOutput")
        dbg_gt = nc.dram_tensor("dbg_gt", [128, KO, NCH, C], BF16, kind="ExternalOutput")
        dbg_ds = nc.dram_tensor("dbg_ds", [128, KO, NCH, HD], BF16, kind="ExternalOutput")
        dbg_oT = nc.dram_tensor("dbg_oT", [128, KO, TT], F32, kind="ExternalOutput")
    else:
        dbg_gT = dbg_hatK = dbg_hatA = dbg_hatQ = None
        dbg_va = dbg_kb = dbg_pnd = dbg_gt = dbg_ds = dbg_oT = None
    ys = nc.dram_tensor("ys", [TT, D], F32, kind="ExternalOutput")

    cc_in = nc.dram_tensor("cc_in", [H * HD, HD], F32)
    cc_out = nc.dram_tensor("cc_out", [2 * H * HD, HD], F32)
    d_ogT = nc.dram_tensor("d_ogT", [128, KO, TT], BF16)

    wq3 = Wq.rearrange("(ko p) f -> p ko f", p=128)
    wk3 = Wk.rearrange("(ko p) f -> p ko f", p=128)
    wv3 = Wv.rearrange("(ko p) f -> p ko f", p=128)
    wo3 = Wo.rearrange("(ko p) f -> p ko f", p=128)
    wg3 = Wgamma.rearrange("(ko p) f -> p ko f", p=128)
    wf13 = Wf1.rearrange("(ko p) f -> p ko f", p=128)
    wog13 = Wog1.rearrange("(ko p) f -> p ko f", p=128)
    x2 = xs.rearrange("(tt p) d -> p tt d", p=128)   # [128, 8, 1024]

    with tile.TileContext(nc) as tc:
        _body(nc, tc, locals())
    nc.compile()
    return nc


def _body(nc, tc, env):
    g = lambda n: env[n]
    xs, ys, cc_in, cc_out = g("xs"), g("ys"), g("cc_in"), g("cc_out")
    Wf2, Wog2, norm_w, parity = g("Wf2"), g("Wog2"), g("norm_w"), g("parity")
    wq3, wk3, wv3, wo3 = g("wq3"), g("wk3"), g("wv3"), g("wo3")
    wg3, wf13, wog13, x2 = g("wg3"), g("wf13"), g("wog13"), g("x2")
    d_ogT = g("d_ogT")
    DBG = g("DBG")

    import contextlib
    ctx = contextlib.ExitStack()
    with ctx:
        ctx.enter_context(nc.allow_low_precision(
            reason="bf16 operands rounded intentionally for PE rate"))
        g0 = ctx.enter_context(tc.tile_pool(name="g0", bufs=1))

        # ---- global constants ----
        identB = g0.tile([128, 128], BF16, tag="identB")
        make_identity(nc, identB)
        ident32 = g0.tile([128, 128], F32, tag="ident32")
        make_identity(nc, ident32)
        cbuild = g0.tile([128, 4], F32, tag="cbuild")
        nc.vector.memset(cbuild, 0.0)
        nc.vector.memset(cbuild[:, 0:1], 1.0)
        nc.vector.memset(cbuild[0:64, 1:2], 1.0)
        nc.vector.memset(cbuild[64:128, 2:3], 1.0)
        ones_l2 = g0.tile([128, 2], F32R, tag="ones_l2")
        nc.scalar.copy(out=ones_l2[:, 0:1], in_=cbuild[:, 1:2])
        nc.scalar.copy(out=ones_l2[:, 1:2], in_=cbuild[:, 2:3])
        onesF = g0.tile([128, 2], F32R, tag="onesF")
        nc.scalar.copy(out=onesF[:, 0:1], in_=cbuild[:, 0:1])
        nc.scalar.copy(out=onesF[:, 1:2], in_=cbuild[:, 0:1])
        c2build = g0.tile([2, 128], F32, tag="c2build")
        nc.vector.memset(c2build, 1.0)
        # ones2T: row p owns col block p (for per-half broadcast of [2,N])
        nc.gpsimd.affine_select(out=c2build, in_=c2build,
                                compare_op=ALU.is_ge, fill=0.0,
                                base=0, pattern=[[1, 128]],
                                channel_multiplier=-64)
        nc.gpsimd.affine_select(out=c2build, in_=c2build,
                                compare_op=ALU.is_ge, fill=0.0,
                                base=63, pattern=[[-1, 128]],
                                channel_multiplier=64)
        ones2T = g0.tile([2, 128], F32R, tag="ones2T")
        nc.scalar.copy(out=ones2T, in_=c2build)
        nc.vector.memset(c2build, 1.0)
        ones2F = g0.tile([2, 128], F32R, tag="ones2F")
        nc.scalar.copy(out=ones2F, in_=c2build)
        # maskWM [128 s, 256]: cols 0:128 strict upper (t>s) for W^T,
        # cols 128:256 inclusive upper (t>=s) for M^T.
        maskWM = g0.tile([128, 256], BF16, tag="maskWM")
        nc.vector.memset(maskWM, 1.0)
        nc.gpsimd.affine_select(
            out=maskWM[:, 0:128], in_=maskWM[:, 0:128],
            compare_op=ALU.is_ge, fill=0.0,
            base=-1, pattern=[[1, 128]], channel_multiplier=-1)
        nc.gpsimd.affine_select(
            out=maskWM[:, 128:256], in_=maskWM[:, 128:256],
            compare_op=ALU.is_ge, fill=0.0,
            base=0, pattern=[[1, 128]], channel_multiplier=-1)
        nw_sb = g0.tile([128, KO], F32, tag="nw_sb")
        nc.sync.dma_start(out=nw_sb, in_=norm_w.rearrange("(ko p) -> p ko", p=128))
        eps_sb = g0.tile([2, 1], F32, tag="eps_sb")
        nc.vector.memset(eps_sb, 1e-5)

        # per-(dim, block/chunk) decay columns
        lamEndE = g0.tile([128, KO, NCH], F32, tag="lamEndE")   # exp(chunk end)
        negmid = g0.tile([128, KO, 16], F32, tag="negmid")      # -g[mid] per block
        kshift = g0.tile([128, KO, 16], F32, tag="kshift")      # kbar factor
        qshift = g0.tile([128, KO, 16], F32, tag="qshift")      # true-hat factor
        crossM = g0.tile([128, KO, NCH], F32, tag="crossM")     # b1->b2 coupling
        gam_t = g0.tile([128, NCH, H], F32, tag="gam_t")        # -sigmoid
        ginv_t = g0.tile([128, NCH, H], F32, tag="ginv_t")      # 1/gamma

        # resident tensors. SBUF is too small to hold P1 working set and
        # P2/P3 products simultaneously, so dead P1 tensors are reused for
        # later-phase outputs via AP views (aliases noted per tile).
        uni = ctx.enter_context(tc.tile_pool(name="uni", bufs=1))
        hatK = uni.tile([128, KO, TT], BF16, tag="hatK")     # P3: oTb
        hatA = uni.tile([128, KO, TT], BF16, tag="hatA")     # P3: Wo slot
        hatQ = uni.tile([128, KO, TT], BF16, tag="hatQ")
        v_t = uni.tile([128, NCH, D], BF16, tag="v_t")
        xTu = uni.tile([128, KO, TT], BF16, tag="xTu")       # P2: pnd_bd
        gTu = uni.tile([128, KO, TT], F32R, tag="gTu")        # P2: oT
        wslot_u = uni.tile([128, KO, D], BF16, tag="wslot_u")  # P2: gt_all
        ds_all = uni.tile([128, KO, NCH, HD], BF16, tag="ds_all")
        wtmp_u = ctx.enter_context(tc.tile_pool(name="wtmpu", bufs=2))

        # ============ P1: projections + hat tensors ============
        with tc.tile_pool(name="tmp", bufs=2) as tmp, \
             tc.tile_pool(name="ktmp", bufs=2) as ktmp_pool, \
             tc.tile_pool(name="small", bufs=1) as small, \
             tc.tile_pool(name="ps1", bufs=3, space="PSUM") as pswide, \
             tc.tile_pool(name="pst1", bufs=2, space="PSUM") as pstp, \
             tc.tile_pool(name="pn1", bufs=1, space="PSUM") as pn_pool, \
             tc.tile_pool(name="bc1", bufs=1, space="PSUM") as bc_pool:
            wtmp_pool = wtmp_u
            xT = xTu
            gT = gTu
            with tc.tile_pool(name="p1e", bufs=1) as p1e, \
                 tc.tile_pool(name="xrp", bufs=1) as xr_pool, \
                 tc.tile_pool(name="spp", bufs=1) as sp_pool:
                gamT = p1e.tile([16, TT], F32, tag="gamT")
                f1T = p1e.tile([64, TT], BF16, tag="f1T")
                og1T = p1e.tile([64, TT], BF16, tag="og1T")
                wgam_sb = p1e.tile([128, KO, H], BF16, tag="wgam_sb")
                wf1_sb = p1e.tile([128, KO, HD], BF16, tag="wf1_sb")
                wog1_sb = p1e.tile([128, KO, HD], BF16, tag="wog1_sb")
                wf2_sb = p1e.tile([64, D], BF16, tag="wf2_sb")
                wog2_sb = p1e.tile([64, D], BF16, tag="wog2_sb")
                wt1 = wtmp_pool.tile([128, D], F32, tag="wtmp")
                nc.sync.dma_start(
                    out=wt1[:, 0:128].rearrange("p (ko f) -> p ko f", ko=KO),
                    in_=wg3)
                nc.sync.dma_start(
                    out=wt1[:, 128:640].rearrange("p (ko f) -> p ko f", ko=KO),
                    in_=wf13)
                nc.vector.tensor_copy(
                    out=wgam_sb,
                    in_=wt1[:, 0:128].rearrange("p (ko f) -> p ko f", ko=KO))
                nc.vector.tensor_copy(
                    out=wf1_sb,
                    in_=wt1[:, 128:640].rearrange("p (ko f) -> p ko f", ko=KO))
                wt2 = wtmp_pool.tile([128, D], F32, tag="wtmp")
                nc.sync.dma_start(
                    out=wt2[:, 0:512].rearrange("p (ko f) -> p ko f", ko=KO),
                    in_=wog13)
                nc.vector.tensor_copy(
                    out=wog1_sb,
                    in_=wt2[:, 0:512].rearrange("p (ko f) -> p ko f", ko=KO))
                wt3 = wtmp_pool.tile([128, D], F32, tag="wtmp")
                nc.sync.dma_start(out=wt3[0:64, :], in_=Wf2[:, :])
                nc.vector.tensor_copy(out=wf2_sb, in_=wt3[0:64, :])
                wt4 = wtmp_pool.tile([128, D], F32, tag="wtmp")
                nc.sync.dma_start(out=wt4[0:64, :], in_=Wog2[:, :])
                nc.vector.tensor_copy(out=wog2_sb, in_=wt4[0:64, :])

                # x -> xT (PE transpose, f32 in -> bf16 out)
                for tt in range(KO):
                    xrow = xr_pool.tile([128, D], F32, tag="xrow")
                    nc.sync.dma_start(out=xrow, in_=x2[:, tt, :])
                    for j in range(KO):
                        pst = pstp.tile([128, 128], F32, tag="pst")
                        nc.tensor.transpose(pst,
                                            xrow[:, j * 128:(j + 1) * 128],
                                            ident32)
                        nc.vector.tensor_copy(
                            out=xT[:, j, tt * 128:(tt + 1) * 128], in_=pst)

                # gamma / f1 / og1 projections (T-layout outputs)
                def proj_small(wap, dout, evac):
                    for tb in range(2):
                        ps = pswide.tile([128, 512], F32, tag="projT")
                        for ko in range(KO):
                            nc.tensor.matmul(
                                ps[:dout, :], wap[:, ko, :],
                                xT[:, ko, tb * 512:(tb + 1) * 512],
                                start=(ko == 0), stop=(ko == KO - 1))
                        evac(ps, tb)

                proj_small(wgam_sb, 16, lambda ps, tb: nc.scalar.activation(
                    out=gamT[:, tb * 512:(tb + 1) * 512], in_=ps[:16, :],
                    func=AF.Sigmoid))
                proj_small(wf1_sb, 64, lambda ps, tb: nc.scalar.copy(
                    out=f1T[:, tb * 512:(tb + 1) * 512], in_=ps[:64, :]))
                proj_small(wog1_sb, 64, lambda ps, tb: nc.scalar.copy(
                    out=og1T[:, tb * 512:(tb + 1) * 512], in_=ps[:64, :]))

                # gam_t (negated) + ginv
                for cch in range(NCH):
                    pst = pstp.tile([128, 128], F32, tag="pst")
                    nc.tensor.transpose(
                        pst[:, 0:16], gamT[:, cch * 128:(cch + 1) * 128],
                        ident32[0:16, 0:16])
                    nc.vector.tensor_scalar_mul(gam_t[:, cch, :],
                                                pst[:, 0:16], -1.0)
                nc.vector.reciprocal_approx_fast(out=ginv_t[:, :, :],
                                                 in_=gam_t[:, :, :])

                # gk: f2 proj -> sigmoid -> ln -> per-64 cumsum -> gT
                for ko in range(KO):
                    for tb in range(2):
                        ps = pswide.tile([128, 512], F32, tag="projT")
                        nc.tensor.matmul(ps,
                                         wf2_sb[:, ko * 128:(ko + 1) * 128],
                                         f1T[:, tb * 512:(tb + 1) * 512],
                                         start=True, stop=True)
                        sp = sp_pool.tile([128, 512], F32, tag="sp")
                        nc.scalar.activation(out=sp, in_=ps, func=AF.Sigmoid)
                        nc.scalar.activation(out=sp, in_=sp, func=AF.Ln)
                        for cc in range(8):
                            b = tb * 8 + cc
                            nc.vector.tensor_tensor_scan(
                                out=gT[:, ko, b * 64:(b + 1) * 64],
                                data0=sp[:, cc * 64:(cc + 1) * 64],
                                data1=sp[:, cc * 64:(cc + 1) * 64],
                                initial=0.0, op0=ALU.add, op1=ALU.bypass)
                    Ee = gT[:, ko, 63::128]     # even-block ends   [128, 8]
                    Eo = gT[:, ko, 127::128]    # odd-block ends
                    Me = gT[:, ko, 31::128]     # even-block mids
                    Mo = gT[:, ko, 95::128]     # odd-block mids
                    nc.vector.tensor_scalar_mul(negmid[:, ko, 0::2], Me, -1.0)
                    nc.vector.tensor_scalar_mul(negmid[:, ko, 1::2], Mo, -1.0)
                    t8 = small.tile([128, 8], F32, tag="t8")
                    nc.vector.tensor_add(out=t8, in0=Ee, in1=Eo)
                    nc.scalar.activation(out=lamEndE[:, ko, :], in_=t8,
                                         func=AF.Exp)
                    nc.vector.tensor_sub(out=t8, in0=Ee, in1=Me)
                    nc.vector.tensor_add(out=t8, in0=t8, in1=Mo)
                    nc.scalar.activation(out=crossM[:, ko, :], in_=t8,
                                         func=AF.Exp)
                    nc.vector.tensor_add(out=t8, in0=Ee, in1=Eo)
                    nc.vector.tensor_sub(out=t8, in0=t8, in1=Me)
                    nc.scalar.activation(out=kshift[:, ko, 0::2], in_=t8,
                                         func=AF.Exp)
                    nc.vector.tensor_sub(out=t8, in0=Eo, in1=Mo)
                    nc.scalar.activation(out=kshift[:, ko, 1::2], in_=t8,
                                         func=AF.Exp)
                    nc.scalar.activation(out=qshift[:, ko, 0::2], in_=Me,
                                         func=AF.Exp)
                    nc.vector.tensor_add(out=t8, in0=Mo, in1=Ee)
                    nc.scalar.activation(out=qshift[:, ko, 1::2], in_=t8,
                                         func=AF.Exp)

                # og2 -> sigmoid -> DRAM
                for ko in range(KO):
                    for tb in range(2):
                        ps = pswide.tile([128, 512], F32, tag="projT")
                        nc.tensor.matmul(ps,
                                         wog2_sb[:, ko * 128:(ko + 1) * 128],
                                         og1T[:, tb * 512:(tb + 1) * 512],
                                         start=True, stop=True)
                        ogt = sp_pool.tile([128, 512], BF16, tag="ogt")
                        nc.scalar.activation(out=ogt, in_=ps, func=AF.Sigmoid)
                        nc.sync.dma_start(
                            out=d_ogT[:, ko, tb * 512:(tb + 1) * 512],
                            in_=ogt)

            def load_wbf(w3ap):
                wslot = wslot_u
                for kk in range(KO):
                    wt = wtmp_pool.tile([128, D], F32, tag="wtmp")
                    nc.sync.dma_start(out=wt, in_=w3ap[:, kk, :])
                    nc.vector.tensor_copy(out=wslot[:, kk, :], in_=wt)
                return wslot

            # v projection (token layout) - x stationary, 2 MMs per LDW
            wslot = load_wbf(wv3)
            for tt in range(NCH):
                psA = pswide.tile([128, 512], F32, tag="projT")
                psB = pswide.tile([128, 512], F32, tag="projT")
                for kk in range(KO):
                    lhs = xT[:, kk, tt * 128:(tt + 1) * 128]
                    nc.tensor.matmul(psA, lhs, wslot[:, kk, 0:512],
                                     start=(kk == 0), stop=(kk == KO - 1))
                    nc.tensor.matmul(psB, lhs, wslot[:, kk, 512:1024],
                                     start=(kk == 0), stop=(kk == KO - 1))
                nc.vector.tensor_copy(out=v_t[:, tt, 0:512], in_=psA)
                nc.vector.tensor_copy(out=v_t[:, tt, 512:1024], in_=psB)

            # q projection -> hatQ
            wslot = load_wbf(wq3)
            for ko in range(KO):
                psA = pswide.tile([128, 512], F32, tag="projT")
                psB = pswide.tile([128, 512], F32, tag="projT")
                for kk in range(KO):
                    lhs = wslot[:, kk, ko * 128:(ko + 1) * 128]
                    nc.tensor.matmul(psA, lhs, xT[:, kk, 0:512],
                                     start=(kk == 0), stop=(kk == KO - 1))
                    nc.tensor.matmul(psB, lhs, xT[:, kk, 512:1024],
                                     start=(kk == 0), stop=(kk == KO - 1))
                for tb, ps in ((0, psA), (1, psB)):
                    qs = tmp.tile([128, 512], F32, tag="qs")
                    nc.scalar.activation(out=qs, in_=ps, func=AF.Silu)
                    eg = tmp.tile([128, 512], F32, tag="eg")
                    for cc in range(8):
                        b = tb * 8 + cc
                        nc.scalar.activation(
                            out=eg[:, cc * 64:(cc + 1) * 64],
                            in_=gT[:, ko, b * 64:(b + 1) * 64],
                            func=AF.Exp, bias=negmid[:, ko, b:b + 1])
                    nc.vector.tensor_mul(
                        out=hatQ[:, ko, tb * 512:(tb + 1) * 512],
                        in0=qs, in1=eg)

            # k projection -> hatK, hatA
            wslot = load_wbf(wk3)
            for ko in range(KO):
                psA = pswide.tile([128, 512], F32, tag="projT")
                psB = pswide.tile([128, 512], F32, tag="projT")
                for kk in range(KO):
                    lhs = wslot[:, kk, ko * 128:(ko + 1) * 128]
                    nc.tensor.matmul(psA, lhs, xT[:, kk, 0:512],
                                     start=(kk == 0), stop=(kk == KO - 1))
                    nc.tensor.matmul(psB, lhs, xT[:, kk, 512:1024],
                                     start=(kk == 0), stop=(kk == KO - 1))
                for tb, ps in ((0, psA), (1, psB)):
                    ks = ktmp_pool.tile([128, 512], F32, tag="ks")
                    nc.scalar.activation(out=ks, in_=ps, func=AF.Silu)
                    k2 = tmp.tile([128, 512], F32R, tag="k2r")
                    nc.vector.tensor_mul(out=k2, in0=ks, in1=ks)
                    pn = pn_pool.tile([2, 512], F32, tag="pn")
                    nc.tensor.matmul(pn, ones_l2, k2, start=True, stop=True)
                    nrm = small.tile([2, 512], F32, tag="nrm")
                    nc.vector.tensor_scalar_max(nrm, pn, 1e-24)
                    rp = small.tile([2, 512], F32, tag="rp")
                    nc.vector.reciprocal_approx_fast(out=rp, in_=nrm)
                    rinv = small.tile([2, 512], F32R, tag="rinv")
                    nc.scalar.activation(out=rinv, in_=rp, func=AF.Sqrt)
                    bcn = bc_pool.tile([128, 512], F32, tag="bc")
                    nc.tensor.matmul(bcn, ones2T, rinv, start=True,
                                     stop=True)
                    kn = ktmp_pool.tile([128, 512], F32, tag="kn")
                    nc.vector.tensor_mul(out=kn, in0=ks, in1=bcn)
                    # hatK = kn * exp(-(g - mid))
                    egn = tmp.tile([128, 512], F32, tag="eg")
                    for cc in range(8):
                        b = tb * 8 + cc
                        nc.scalar.activation(
                            out=egn[:, cc * 64:(cc + 1) * 64],
                            in_=gT[:, ko, b * 64:(b + 1) * 64],
                            func=AF.Exp, scale=-1.0,
                            bias=gT[:, ko, b * 64 + 31:b * 64 + 32])
                    nc.vector.tensor_mul(
                        out=hatK[:, ko, tb * 512:(tb + 1) * 512],
                        in0=kn, in1=egn)
                    # hatA = kn * exp(2g - gprev - mid)
                    twog = tmp.tile([128, 512], F32, tag="twog")
                    gsl = gT[:, ko, tb * 512:(tb + 1) * 512]
                    nc.vector.tensor_scalar_mul(twog, gsl, 2.0)
                    for cc in range(8):
                        sl = slice(cc * 64 + 1, (cc + 1) * 64)
                        slp = slice(cc * 64, (cc + 1) * 64 - 1)
                        nc.vector.tensor_sub(out=twog[:, sl], in0=twog[:, sl],
                                             in1=gsl[:, slp])
                    for cc in range(8):
                        b = tb * 8 + cc
                        nc.scalar.activation(
                            out=twog[:, cc * 64:(cc + 1) * 64],
                            in_=twog[:, cc * 64:(cc + 1) * 64],
                            func=AF.Exp, bias=negmid[:, ko, b:b + 1])
                    nc.vector.tensor_mul(
                        out=hatA[:, ko, tb * 512:(tb + 1) * 512],
                        in0=twog, in1=kn)

            if DBG:
                nc.sync.dma_start(out=g("dbg_gT")[:, :, :], in_=gT)
                nc.sync.dma_start(out=g("dbg_hatK")[:, :, :], in_=hatK)
                nc.sync.dma_start(out=g("dbg_hatA")[:, :, :], in_=hatA)
                nc.sync.dma_start(out=g("dbg_hatQ")[:, :, :], in_=hatQ)

        # ============ P2: phase A per (chunk, head) ============
        # P2/P3 products alias dead P1 tensors (same byte size per tile)
        pnd_bd = xTu.rearrange("p ko (nc q) -> p ko nc q", q=128)
        gt_all = wslot_u.rearrange("p ko (nc q) -> p ko nc q", q=C)
        oT = gTu
        nc.vector.memset(pnd_bd, 0.0)

        with tc.tile_pool(name="vab", bufs=3) as va_pool, \
             tc.tile_pool(name="kbb", bufs=3) as kb_pool, \
             tc.tile_pool(name="sc", bufs=3) as sc_pool, \
             tc.tile_pool(name="wm", bufs=4) as wm_pool, \
             tc.tile_pool(name="zp", bufs=8) as z_pool, \
             tc.tile_pool(name="psg", bufs=2, space="PSUM") as psg_pool, \
             tc.tile_pool(name="psx", bufs=2, space="PSUM") as psx_pool, \
             tc.tile_pool(name="psf", bufs=2, space="PSUM") as psf_pool, \
             tc.tile_pool(name="pstr", bufs=2, space="PSUM") as pstr_pool:
            for c in range(NCH):
                csl = slice(c * C, (c + 1) * C)
                s1 = slice(c * C, c * C + 64)
                s2 = slice(c * C + 64, (c + 1) * C)
                va_buf = va_pool.tile([128, H, C], BF16, tag="va")
                kb_buf = kb_pool.tile([128, H, HD], BF16, tag="kb")
                k1c_all = sc_pool.tile([128, KO, 64], BF16, tag="k1c")
                for ko in range(KO):
                    # scaled K1 for the b1->b2 coupling
                    nc.gpsimd.tensor_scalar_mul(
                        k1c_all[:, ko, :], hatK[:, ko, s1],
                        crossM[:, ko, c:c + 1])
                    # true A^T columns for the va rhs
                    atr = sc_pool.tile([128, 128], BF16, tag="atr")
                    nc.gpsimd.tensor_scalar_mul(
                        atr[:, 0:64], hatA[:, ko, s1],
                        qshift[:, ko, 2 * c:2 * c + 1])
                    nc.gpsimd.tensor_scalar_mul(
                        atr[:, 64:128], hatA[:, ko, s2],
                        qshift[:, ko, 2 * c + 1:2 * c + 2])
                    psT = pstr_pool.tile([128, 128], BF16, tag="psT")
                    nc.tensor.transpose(psT, atr, identB)
                    nc.scalar.copy(out=va_buf[:, 2 * ko, 64:128],
                                   in_=psT[:, 0:64])
                    nc.scalar.copy(out=va_buf[:, 2 * ko + 1, 64:128],
                                   in_=psT[:, 64:128])
                    # va_V = v / gamma
                    for par in range(2):
                        h = 2 * ko + par
                        nc.gpsimd.tensor_scalar_mul(
                            va_buf[:, h, 0:64],
                            v_t[:, c, h * 64:(h + 1) * 64],
                            ginv_t[:, c, h:h + 1])
                    # kbar = hatK * kshift -> transpose -> gamma scale
                    kbt = sc_pool.tile([128, 128], BF16, tag="kbt")
                    nc.gpsimd.tensor_scalar_mul(
                        kbt[:, 0:64], hatK[:, ko, s1],
                        kshift[:, ko, 2 * c:2 * c + 1])
                    nc.gpsimd.tensor_scalar_mul(
                        kbt[:, 64:128], hatK[:, ko, s2],
                        kshift[:, ko, 2 * c + 1:2 * c + 2])
                    psK = pstr_pool.tile([128, 128], BF16, tag="psT")
                    nc.tensor.transpose(psK, kbt, identB)
                    nc.vector.tensor_scalar_mul(
                        kb_buf[:, 2 * ko, :], psK[:, 0:64],
                        gam_t[:, c, 2 * ko:2 * ko + 1])
                    nc.vector.tensor_scalar_mul(
                        kb_buf[:, 2 * ko + 1, :], psK[:, 64:128],
                        gam_t[:, c, 2 * ko + 1:2 * ko + 2])
                if DBG:
                    nc.sync.dma_start(out=g("dbg_va")[:, c, :, :], in_=va_buf)
                    nc.sync.dma_start(out=g("dbg_kb")[:, c, :, :], in_=kb_buf)

                for h in range(H):
                    hb = (h % 2) * 64
                    ko = h // 2
                    hsl = slice(hb, hb + 64)
                    kS1 = hatK[hsl, ko, s1]
                    kS2 = hatK[hsl, ko, s2]
                    k1c = k1c_all[hsl, ko, :]
                    aS1 = hatA[hsl, ko, s1]
                    aS2 = hatA[hsl, ko, s2]
                    qS1 = hatQ[hsl, ko, s1]
                    qS2 = hatQ[hsl, ko, s2]
                    psg = psg_pool.tile([128, 256], F32, tag="psg")
                    nc.tensor.matmul(psg[0:64, 0:64], kS1, aS1,
                                     start=True, stop=True,
                                     tile_position=(hb, 0))
                    nc.tensor.matmul(psg[0:64, 128:192], kS1, qS1,
                                     start=True, stop=True,
                                     tile_position=(hb, 0))
                    nc.tensor.matmul(psg[0:64, 64:128], k1c, aS2,
                                     start=True, stop=True,
                                     tile_position=(hb, 0))
                    nc.tensor.matmul(psg[0:64, 192:256], k1c, qS2,
                                     start=True, stop=True,
                                     tile_position=(hb, 0))
                    nc.tensor.matmul(psg[64:128, 64:128], kS2, aS2,
                                     start=True, stop=True,
                                     tile_position=(hb, 64))
                    nc.tensor.matmul(psg[64:128, 192:256], kS2, qS2,
                                     start=True, stop=True,
                                     tile_position=(hb, 64))
                    # fillers: finite garbage under the mask
                    nc.tensor.matmul(psg[64:128, 0:64], kS2, aS1,
                                     start=True, stop=True,
                                     tile_position=(hb, 64))
                    nc.tensor.matmul(psg[64:128, 128:192], kS2, qS1,
                                     start=True, stop=True,
                                     tile_position=(hb, 64))
                    wm = wm_pool.tile([128, 256], BF16, tag="wm")
                    nc.vector.scalar_tensor_tensor(
                        out=wm, in0=psg, scalar=gam_t[:, c, h:h + 1],
                        in1=maskWM, op0=ALU.mult, op1=ALU.mult)
                    va = va_buf[:, h, :]
                    zc = va
                    for it in range(NEU):
                        psx = psx_pool.tile([128, 128], F32, tag="psx")
                        nc.tensor.matmul(psx, wm[:, 0:128], zc,
                                         start=True, stop=True)
                        zn = z_pool.tile([128, 128], BF16, tag="zn")
                        nc.vector.tensor_add(out=zn, in0=psx, in1=va)
                        zc = zn
                    kb = kb_buf[:, h, :]
                    psf = psf_pool.tile([128, 384], F32, tag="psf")
                    nc.tensor.matmul(psf[hsl, 0:64], zc[:, 64:128], kb,
                                     start=True, stop=True,
                                     tile_position=(0, hb))
                    nc.tensor.matmul(psf[hsl, 64:192], zc[:, 64:128],
                                     wm[:, 128:256],
                                     start=True, stop=True,
                                     tile_position=(0, hb))
                    nc.tensor.matmul(psf[hsl, 192:320], zc[:, 0:64],
                                     wm[:, 128:256],
                                     start=True, stop=True,
                                     tile_position=(0, hb))
                    nc.tensor.matmul(psf[hsl, 320:384], kb, zc[:, 0:64],
                                     start=True, stop=True,
                                     tile_position=(0, hb))
                    nc.scalar.copy(out=pnd_bd[hsl, ko, c, hb:hb + 64],
                                   in_=psf[hsl, 0:64])
                    nc.scalar.copy(out=gt_all[hsl, ko, c, :],
                                   in_=psf[hsl, 64:192])
                    nc.vector.tensor_copy(out=oT[hsl, ko, csl],
                                          in_=psf[hsl, 192:320])
                    nc.vector.tensor_copy(out=ds_all[hsl, ko, c, :],
                                          in_=psf[hsl, 320:384])

        if DBG:
            nc.sync.dma_start(out=g("dbg_pnd")[:, :, :, :], in_=pnd_bd)
            nc.sync.dma_start(out=g("dbg_gt")[:, :, :, :], in_=gt_all)
            nc.sync.dma_start(out=g("dbg_ds")[:, :, :, :], in_=ds_all)

        # ============ P3: scans, output, layernorm, Wo ============
        with tc.tile_pool(name="p3", bufs=1) as p3, \
             tc.tile_pool(name="q3", bufs=3) as q3_pool, \
             tc.tile_pool(name="tmp3", bufs=2) as tmp3, \
             tc.tile_pool(name="sb3", bufs=2) as sb3_pool, \
             tc.tile_pool(name="psS", bufs=2, space="PSUM") as psS_pool, \
             tc.tile_pool(name="psO", bufs=2, space="PSUM") as psO_pool, \
             tc.tile_pool(name="ln3", bufs=2, space="PSUM") as ppool3, \
             tc.tile_pool(name="ps3", bufs=2, space="PSUM") as pswide3:
            scur = p3.tile([128, KO, HD], F32, tag="scur")
            nc.vector.memset(scur[:, :, :], 0.0)

            def scan_update(c, sbf):
                psS = psS_pool.tile([128, 512], F32, tag="psS")
                for ko in range(KO):
                    nc.tensor.matmul(psS[:, ko * 64:(ko + 1) * 64],
                                     pnd_bd[:, ko, c, :], sbf[:, ko, :],
                                     start=True, stop=True)
                for ko in range(KO):
                    nc.vector.scalar_tensor_tensor(
                        out=scur[:, ko, :], in0=scur[:, ko, :],
                        scalar=lamEndE[:, ko, c:c + 1],
                        in1=psS[:, ko * 64:(ko + 1) * 64],
                        op0=ALU.mult, op1=ALU.add)
                nc.vector.tensor_add(out=scur[:, :, :], in0=scur[:, :, :],
                                     in1=ds_all[:, :, c, :])

            # pass 1: local final state
            for c in range(NCH):
                sbf = sb3_pool.tile([128, KO, HD], BF16, tag="sbf")
                nc.vector.tensor_copy(out=sbf[:, :, :], in_=scur[:, :, :])
                scan_update(c, sbf)

            # AllGather pair exchange + parity mask
            cin3 = cc_in.rearrange("(ko p) f -> p ko f", p=128)
            cout3 = cc_out.rearrange("(r ko p) f -> r p ko f", p=128, r=2)
            nc.sync.dma_start(out=cin3, in_=scur)
            nc.gpsimd.collective_compute(
                "AllGather", ALU.bypass,
                replica_groups=[[0, 1], [2, 3], [4, 5], [6, 7]],
                ins=[cc_in[:, :]], outs=[cc_out[:, :]])
            sinit = p3.tile([128, KO, HD], F32, tag="sinit")
            nc.sync.dma_start(out=sinit, in_=cout3[0])
            par_col = p3.tile([128, 1], F32, tag="par_col")
            nc.sync.dma_start(out=par_col,
                              in_=parity[0:1, 0:1].to_broadcast((128, 1)))
            nc.vector.tensor_scalar_mul(scur[:, :, :], sinit[:, :, :],
                                        par_col)

            # pass 2: outputs + scan
            sbd_z = p3.tile([128, KO, 128], BF16, tag="sbd")
            nc.vector.memset(sbd_z[:, :, :], 0.0)
            for c in range(NCH):
                csl = slice(c * C, (c + 1) * C)
                sbf = sb3_pool.tile([128, KO, HD], BF16, tag="sbf")
                nc.vector.tensor_copy(out=sbf[:, :, :], in_=scur[:, :, :])
                # block-diag copy for the output matmuls
                nc.gpsimd.tensor_scalar_mul(sbd_z[0:64, :, 0:64],
                                            sbf[0:64, :, :], 1.0)
                nc.gpsimd.tensor_scalar_mul(sbd_z[64:128, :, 64:128],
                                            sbf[64:128, :, :], 1.0)
                for ko in range(KO):
                    # true Qglob^T slice: hatQ scaled by qshift per 64-block
                    qtm = q3_pool.tile([128, 128], BF16, tag="qtm")
                    nc.gpsimd.tensor_scalar_mul(
                        qtm[:, 0:64], hatQ[:, ko, c * C:c * C + 64],
                        qshift[:, ko, 2 * c:2 * c + 1])
                    nc.gpsimd.tensor_scalar_mul(
                        qtm[:, 64:128], hatQ[:, ko, c * C + 64:(c + 1) * C],
                        qshift[:, ko, 2 * c + 1:2 * c + 2])
                    psO = psO_pool.tile([128, 128], F32, tag="psO")
                    nc.tensor.matmul(psO, sbd_z[:, ko, :],
                                     gt_all[:, ko, c, :],
                                     start=True, stop=False)
                    nc.tensor.matmul(psO, sbd_z[:, ko, :], qtm,
                                     start=False, stop=True)
                    nc.vector.tensor_add(out=oT[:, ko, csl],
                                         in0=oT[:, ko, csl], in1=psO)
                if c < NCH - 1:
                    scan_update(c, sbf)

            # output gate
            for ko in range(KO):
                for tb in range(2):
                    ogt = tmp3.tile([128, 512], BF16, tag="ogt3")
                    nc.sync.dma_start(
                        out=ogt, in_=d_ogT[:, ko, tb * 512:(tb + 1) * 512])
                    nc.vector.tensor_mul(
                        out=oT[:, ko, tb * 512:(tb + 1) * 512],
                        in0=oT[:, ko, tb * 512:(tb + 1) * 512], in1=ogt)

            # layernorm stats (feature dim = partitions x ko)
            oTb = hatK   # dead after P2; reused as normalized-output buffer
            stat_mu = p3.tile([2, TT], F32R, tag="stat_mu")
            stat_rs = p3.tile([2, TT], F32R, tag="stat_rs")
            for tb in range(2):
                tsl = slice(tb * 512, (tb + 1) * 512)
                psm = ppool3.tile([2, 512], F32, tag="acc")
                for ko in range(KO):
                    nc.tensor.matmul(psm, onesF, oT[:, ko, tsl],
                                     start=(ko == 0), stop=(ko == KO - 1))
                # both rows hold the full-D sum; fold the K=2 bcast double
                nc.vector.tensor_scalar_mul(stat_mu[:, tsl], psm, 0.5 / D)
                ps2 = ppool3.tile([2, 512], F32, tag="acc")
                for ko in range(KO):
                    o2 = tmp3.tile([128, 512], F32R, tag="o2")
                    nc.vector.tensor_mul(out=o2, in0=oT[:, ko, tsl],
                                         in1=oT[:, ko, tsl])
                    nc.tensor.matmul(ps2, onesF, o2,
                                     start=(ko == 0), stop=(ko == KO - 1))
                msq = tmp3.tile([2, 512], F32, tag="msq")
                nc.vector.tensor_scalar_mul(msq, ps2, 1.0 / D)
                mu2 = tmp3.tile([2, 512], F32, tag="mu2")
                nc.vector.tensor_mul(out=mu2, in0=stat_mu[:, tsl],
                                     in1=stat_mu[:, tsl])
                var = tmp3.tile([2, 512], F32, tag="var")
                nc.vector.scalar_tensor_tensor(
                    out=var, in0=mu2, scalar=-4.0, in1=msq,
                    op0=ALU.mult, op1=ALU.add)
                nc.scalar.activation(out=var, in_=var, func=AF.Sqrt,
                                     bias=eps_sb)
                rs = tmp3.tile([2, 512], F32, tag="rs")
                nc.vector.reciprocal_approx_fast(out=rs, in_=var)
                nc.vector.tensor_scalar_mul(stat_rs[:, tsl], rs, 0.5)
            for tb in range(2):
                tsl = slice(tb * 512, (tb + 1) * 512)
                bmu = pswide3.tile([128, 512], F32, tag="projT")
                nc.tensor.matmul(bmu, ones2F, stat_mu[:, tsl],
                                 start=True, stop=True)
                brs = pswide3.tile([128, 512], F32, tag="projT")
                nc.tensor.matmul(brs, ones2F, stat_rs[:, tsl],
                                 start=True, stop=True)
                for ko in range(KO):
                    osl = oT[:, ko, tsl]
                    t1 = tmp3.tile([128, 512], F32, tag="t1f")
                    nc.vector.tensor_sub(out=t1, in0=osl, in1=bmu)
                    t2 = tmp3.tile([128, 512], F32, tag="t2f")
                    nc.vector.tensor_mul(out=t2, in0=t1, in1=brs)
                    nc.vector.tensor_scalar_mul(
                        oTb[:, ko, tsl], t2, nw_sb[:, ko:ko + 1])

            if DBG:
                nc.sync.dma_start(out=g("dbg_oT")[:, :, :], in_=oT)

            # final Wo (staged into hatA, dead after P2)
            wslot = hatA
            for kk in range(KO):
                wt = wtmp_u.tile([128, D], F32, tag="wtmp")
                nc.sync.dma_start(out=wt, in_=wo3[:, kk, :])
                nc.vector.tensor_copy(out=wslot[:, kk, :], in_=wt)
            y2 = ys.rearrange("(tt p) d -> p tt d", p=128)
            for tt in range(NCH):
                psA = pswide3.tile([128, 512], F32, tag="projT")
                psB = pswide3.tile([128, 512], F32, tag="projT")
                for kk in range(KO):
                    lhs = oTb[:, kk, tt * 128:(tt + 1) * 128]
                    nc.tensor.matmul(psA, lhs, wslot[:, kk, 0:512],
                                     start=(kk == 0), stop=(kk == KO - 1))
                    nc.tensor.matmul(psB, lhs, wslot[:, kk, 512:1024],
                                     start=(kk == 0), stop=(kk == KO - 1))
                yrow = tmp3.tile([128, D], F32, tag="yrow")
                nc.vector.tensor_copy(out=yrow[:, 0:512], in_=psA)
                nc.vector.tensor_copy(out=yrow[:, 512:1024], in_=psB)
                nc.sync.dma_start(out=y2[:, tt, :], in_=yrow)


_NC = None


def _get_nc():
    global _NC
    if _NC is None:
        _NC = build()
    return _NC


def kernel(**inputs):
    nc = _get_nc()
    x = np.ascontiguousarray(np.asarray(inputs["x"], dtype=np.float32))
    names = ["Wq", "Wk", "Wv", "Wgamma", "Wf1", "Wf2", "Wog1", "Wog2",
             "norm_w", "Wo"]
    w = {n: np.ascontiguousarray(np.asarray(inputs[n], np.float32))
         for n in names}
    in_maps = []
    for core in range(8):
        b, half = core // 2, core % 2
        m = dict(w)
        m["xs"] = np.ascontiguousarray(x[b, half * TT:(half + 1) * TT, :])
        m["parity"] = np.array([[float(half)]], np.float32)
        in_maps.append(m)
    res = run_bass_kernel_spmd(nc, in_maps, core_ids=list(range(8)))
    out = np.empty((B, T, D), np.float32)
    for core in range(8):
        b, half = core // 2, core % 2
        out[b, half * TT:(half + 1) * TT, :] = res.results[core]["ys"]
    return out


# revision 29
# speedup vs baseline: 2.7906x; 1.4796x over previous
"""PolarRnn (gated DPLR delta-rule linear RNN) Trainium2 Bass kernel.

Sharding: 8 cores = (batch b, sequence-half s). Each core processes 1024
tokens x full D=1024 (all 16 heads). Cross-core dependency: recurrent state
at the half boundary via a pair AllGather + parity-masked second scan pass.

Algorithm: chunk-parallel delta rule, chunk C=128 (one partition block).
Numerics: decay exponents are kept 64-block-local and mid-shifted (range
e^{+-23}) so all stored "hat" factors and all PE products stay finite in
bf16/fp32; cross-block couplings are restored with per-partition exp columns
(crossM/kshift/qshift).

Per (head h, chunk c), blocks b1/b2, token gamma < 0:
  W^T/M^T via 8 quadrant matmuls of hatK/hatA/hatQ (+ scaled K1c for the
  b1->b2 coupling); wm = (psg * gamma_s) * mask  [one fused STT]
  z-substitution: solve z = va' + (W Gamma) z by NEU Neumann iterations,
  va' = [V/gamma | A_true^T], so no per-iteration gamma scaling is needed.
  pnd = z_a^T Kbar', gt = z_a^T (Gamma M^T), oloc = z_u^T (Gamma M^T),
  dS = Kbar'^T z_u, with Kbar' = Gamma Kbar.
Scan (per chunk, all 16 heads batched): S' = lamEnd*S + pnd_bd^T S + dS;
outputs O^T = oloc + S^T(gt + Qglob^T) via block-diag sbf matmuls.
"""
import numpy as np

import concourse.bass as bass
import concourse.bacc as bacc
import concourse.mybir as mybir
import concourse.tile as tile
from concourse.masks import make_identity
from concourse.bass_utils import run_bass_kernel_spmd

F32 = mybir.dt.float32
F32R = mybir.dt.float32r
BF16 = mybir.dt.bfloat16

B, T, D, H, HD = 4, 2048, 1024, 16, 64
TT = 1024          # tokens per core
C = 128            # chunk (= partition block)
NCH = TT // C      # 8 chunks
KO = D // 128      # 8 feature blocks (2 heads each)
NEU = 2            # Neumann iterations
AF = mybir.ActivationFunctionType
ALU = mybir.AluOpType


def r(ap):
    return ap.bitcast(F32R)


def build():
    nc = bacc.Bacc("TRN2", target_bir_lowering=False)
    xs = nc.dram_tensor("xs", [TT, D], F32, kind="ExternalInput")
    Wq = nc.dram_tensor("Wq", [D, D], F32, kind="ExternalInput")
    Wk = nc.dram_tensor("Wk", [D, D], F32, kind="ExternalInput")
    Wv = nc.dram_tensor("Wv", [D, D], F32, kind="ExternalInput")
    Wgamma = nc.dram_tensor("Wgamma", [D, H], F32, kind="ExternalInput")
    Wf1 = nc.dram_tensor("Wf1", [D, HD], F32, kind="ExternalInput")
    Wf2 = nc.dram_tensor("Wf2", [HD, D], F32, kind="ExternalInput")
    Wog1 = nc.dram_tensor("Wog1", [D, HD], F32, kind="ExternalInput")
    Wog2 = nc.dram_tensor("Wog2", [HD, D], F32, kind="ExternalInput")
    norm_w = nc.dram_tensor("norm_w", [D], F32, kind="ExternalInput")
    Wo = nc.dram_tensor("Wo", [D, D], F32, kind="ExternalInput")
    parity = nc.dram_tensor("parity", [1, 1], F32, kind="ExternalInput")
    import os
    DBG = os.environ.get("KDBG", "0") == "1"
    if DBG:
        dbg_gT = nc.dram_tensor("dbg_gT", [128, KO, TT], F32, kind="ExternalOutput")
        dbg_hatK = nc.dram_tensor("dbg_hatK", [128, KO, TT], BF16, kind="ExternalOutput")
        dbg_hatA = nc.dram_tensor("dbg_hatA", [128, KO, TT], BF16, kind="ExternalOutput")
        dbg_hatQ = nc.dram_tensor("dbg_hatQ", [128, KO, TT], BF16, kind="ExternalOutput")
        dbg_va = nc.dram_tensor("dbg_va", [128, NCH, H, C], BF16, kind="ExternalOutput")
        dbg_kb = nc.dram_tensor("dbg_kb", [128, NCH, H, HD], BF16, kind="ExternalOutput")
        dbg_pnd = nc.dram_tensor("dbg_pnd", [128, KO, NCH, 128], BF16, kind="ExternalOutput")
        dbg_gt = nc.dram_tensor("dbg_gt", [128, KO, NCH, C], BF16, kind="ExternalOutput")
        dbg_ds = nc.dram_tensor("dbg_ds", [128, KO, NCH, HD], BF16, kind="ExternalOutput")
        dbg_oT = nc.dram_tensor("dbg_oT", [128, KO, TT], F32, kind="ExternalOutput")
    else:
        dbg_gT = dbg_hatK = dbg_hatA = dbg_hatQ = None
        dbg_va = dbg_kb = dbg_pnd = dbg_gt = dbg_ds = dbg_oT = None
    ys = nc.dram_tensor("ys", [TT, D], F32, kind="ExternalOutput")

    cc_in = nc.dram_tensor("cc_in", [H * HD, HD], F32)
    cc_out = nc.dram_tensor("cc_out", [2 * H * HD, HD], F32)
    d_ogT = nc.dram_tensor("d_ogT", [128, KO, TT], BF16)

    wq3 = Wq.rearrange("(ko p) f -> p ko f", p=128)
    wk3 = Wk.rearrange("(ko p) f -> p ko f", p=128)
    wv3 = Wv.rearrange("(ko p) f -> p ko f", p=128)
    wo3 = Wo.rearrange("(ko p) f -> p ko f", p=128)
    wg3 = Wgamma.rearrange("(ko p) f -> p ko f", p=128)
    wf13 = Wf1.rearrange("(ko p) f -> p ko f", p=128)
    wog13 = Wog1.rearrange("(ko p) f -> p ko f", p=128)
    x2 = xs.rearrange("(tt p) d -> p tt d", p=128)   # [128, 8, 1024]

    with tile.TileContext(nc) as tc:
        _body(nc, tc, locals())
    nc.compile()
    return nc


def _body(nc, tc, env):
    g = lambda n: env[n]
    xs, ys, cc_in, cc_out = g("xs"), g("ys"), g("cc_in"), g("cc_out")
    Wf2, Wog2, norm_w, parity = g("Wf2"), g("Wog2"), g("norm_w"), g("parity")
    wq3, wk3, wv3, wo3 = g("wq3"), g("wk3"), g("wv3"), g("wo3")
    wg3, wf13, wog13, x2 = g("wg3"), g("wf13"), g("wog13"), g("x2")
    d_ogT = g("d_ogT")
    DBG = g("DBG")

    import contextlib
    ctx = contextlib.ExitStack()
    with ctx:
        ctx.enter_context(nc.allow_low_precision(
            reason="bf16 operands rounded intentionally for PE rate"))
        g0 = ctx.enter_context(tc.tile_pool(name="g0", bufs=1))

        # ---- global constants ----
        identB = g0.tile([128, 128], BF16, tag="identB")
        make_identity(nc, identB)
        ident32 = g0.tile([128, 128], F32, tag="ident32")
        make_identity(nc, ident32)
        cbuild = g0.tile([128, 4], F32, tag="cbuild")
        nc.vector.memset(cbuild, 0.0)
        nc.vector.memset(cbuild[:, 0:1], 1.0)
        nc.vector.memset(cbuild[0:64, 1:2], 1.0)
        nc.vector.memset(cbuild[64:128, 2:3], 1.0)
        ones_l2 = g0.tile([128, 2], F32R, tag="ones_l2")
        nc.scalar.copy(out=ones_l2[:, 0:1], in_=cbuild[:, 1:2])
        nc.scalar.copy(out=ones_l2[:, 1:2], in_=cbuild[:, 2:3])
        onesF = g0.tile([128, 2], F32R, tag="onesF")
        nc.scalar.copy(out=onesF[:, 0:1], in_=cbuild[:, 0:1])
        nc.scalar.copy(out=onesF[:, 1:2], in_=cbuild[:, 0:1])
        c2build = g0.tile([2, 128], F32, tag="c2build")
        nc.vector.memset(c2build, 1.0)
        # ones2T: row p owns col block p (for per-half broadcast of [2,N])
        nc.gpsimd.affine_select(out=c2build, in_=c2build,
                                compare_op=ALU.is_ge, fill=0.0,
                                base=0, pattern=[[1, 128]],
                                channel_multiplier=-64)
        nc.gpsimd.affine_select(out=c2build, in_=c2build,
                                compare_op=ALU.is_ge, fill=0.0,
                                base=63, pattern=[[-1, 128]],
                                channel_multiplier=64)
        ones2T = g0.tile([2, 128], F32R, tag="ones2T")
        nc.scalar.copy(out=ones2T, in_=c2build)
        nc.vector.memset(c2build, 1.0)
        ones2F = g0.tile([2, 128], F32R, tag="ones2F")
        nc.scalar.copy(out=ones2F, in_=c2build)
        # maskWM [128 s, 256]: cols 0:128 strict upper (t>s) for W^T,
        # cols 128:256 inclusive upper (t>=s) for M^T.
        maskWM = g0.tile([128, 256], BF16, tag="maskWM")
        nc.vector.memset(maskWM, 1.0)
        nc.gpsimd.affine_select(
            out=maskWM[:, 0:128], in_=maskWM[:, 0:128],
            compare_op=ALU.is_ge, fill=0.0,
            base=-1, pattern=[[1, 128]], channel_multiplier=-1)
        nc.gpsimd.affine_select(
            out=maskWM[:, 128:256], in_=maskWM[:, 128:256],
            compare_op=ALU.is_ge, fill=0.0,
            base=0, pattern=[[1, 128]], channel_multiplier=-1)
        nw_sb = g0.tile([128, KO], F32, tag="nw_sb")
        nc.sync.dma_start(out=nw_sb, in_=norm_w.rearrange("(ko p) -> p ko", p=128))
        eps_sb = g0.tile([2, 1], F32, tag="eps_sb")
        nc.vector.memset(eps_sb, 1e-5)

        # per-(dim, block/chunk) decay columns
        lamEndE = g0.tile([128, KO, NCH], F32, tag="lamEndE")   # exp(chunk end)
        negmid = g0.tile([128, KO, 16], F32, tag="negmid")      # -g[mid] per block
        kshift = g0.tile([128, KO, 16], F32, tag="kshift")      # kbar factor
        qshift = g0.tile([128, KO, 16], F32, tag="qshift")      # true-hat factor
        crossM = g0.tile([128, KO, NCH], F32, tag="crossM")     # b1->b2 coupling
        gam_t = g0.tile([128, NCH, H], F32, tag="gam_t")        # -sigmoid
        ginv_t = g0.tile([128, NCH, H], F32, tag="ginv_t")      # 1/gamma

        # resident tensors. SBUF is too small to hold P1 working set and
        # P2/P3 products simultaneously, so dead P1 tensors are reused for
        # later-phase outputs via AP views (aliases noted per tile).
        uni = ctx.enter_context(tc.tile_pool(name="uni", bufs=1))
        hatK = uni.tile([128, KO, TT], BF16, tag="hatK")     # P3: oTb
        hatA = uni.tile([128, KO, TT], BF16, tag="hatA")     # P3: Wo slot
        hatQ = uni.tile([128, KO, TT], BF16, tag="hatQ")
        v_t = uni.tile([128, NCH, D], BF16, tag="v_t")
        xTu = uni.tile([128, KO, TT], BF16, tag="xTu")       # P2: pnd_bd
        gTu = uni.tile([128, KO, TT], F32R, tag="gTu")        # P2: oT
        wslot_u = uni.tile([128, KO, D], BF16, tag="wslot_u")  # P2: gt_all
        ds_all = uni.tile([128, KO, NCH, HD], BF16, tag="ds_all")
        wtmp_u = ctx.enter_context(tc.tile_pool(name="wtmpu", bufs=2))

        # ============ P1: projections + hat tensors ============
        with tc.tile_pool(name="tmp", bufs=2) as tmp, \
             tc.tile_pool(name="ktmp", bufs=2) as ktmp_pool, \
             tc.tile_pool(name="small", bufs=1) as small, \
             tc.tile_pool(name="rp2", bufs=2) as rp_pool, \
             tc.tile_pool(name="ps1", bufs=3, space="PSUM") as pswide, \
             tc.tile_pool(name="pst1", bufs=2, space="PSUM") as pstp, \
             tc.tile_pool(name="pn1", bufs=1, space="PSUM") as pn_pool, \
             tc.tile_pool(name="bc1", bufs=1, space="PSUM") as bc_pool:
            wtmp_pool = wtmp_u
            xT = xTu
            gT = gTu
            with tc.tile_pool(name="p1e", bufs=1) as p1e, \
                 tc.tile_pool(name="xrp", bufs=1) as xr_pool, \
                 tc.tile_pool(name="spp", bufs=1) as sp_pool:
                gamT = p1e.tile([16, TT], F32, tag="gamT")
                f1T = p1e.tile([64, TT], BF16, tag="f1T")
                og1T = p1e.tile([64, TT], BF16, tag="og1T")
                wgam_sb = p1e.tile([128, KO, H], BF16, tag="wgam_sb")
                wf1_sb = p1e.tile([128, KO, HD], BF16, tag="wf1_sb")
                wog1_sb = p1e.tile([128, KO, HD], BF16, tag="wog1_sb")
                wf2_sb = p1e.tile([64, D], BF16, tag="wf2_sb")
                wog2_sb = p1e.tile([64, D], BF16, tag="wog2_sb")
                wt1 = wtmp_pool.tile([128, D], F32, tag="wtmp")
                nc.sync.dma_start(
                    out=wt1[:, 0:128].rearrange("p (ko f) -> p ko f", ko=KO),
                    in_=wg3)
                nc.sync.dma_start(
                    out=wt1[:, 128:640].rearrange("p (ko f) -> p ko f", ko=KO),
                    in_=wf13)
                nc.vector.tensor_copy(
                    out=wgam_sb,
                    in_=wt1[:, 0:128].rearrange("p (ko f) -> p ko f", ko=KO))
                nc.vector.tensor_copy(
                    out=wf1_sb,
                    in_=wt1[:, 128:640].rearrange("p (ko f) -> p ko f", ko=KO))
                wt2 = wtmp_pool.tile([128, D], F32, tag="wtmp")
                nc.sync.dma_start(
                    out=wt2[:, 0:512].rearrange("p (ko f) -> p ko f", ko=KO),
                    in_=wog13)
                nc.vector.tensor_copy(
                    out=wog1_sb,
                    in_=wt2[:, 0:512].rearrange("p (ko f) -> p ko f", ko=KO))
                wt3 = wtmp_pool.tile([128, D], F32, tag="wtmp")
                nc.sync.dma_start(out=wt3[0:64, :], in_=Wf2[:, :])
                nc.vector.tensor_copy(out=wf2_sb, in_=wt3[0:64, :])
                wt4 = wtmp_pool.tile([128, D], F32, tag="wtmp")
                nc.sync.dma_start(out=wt4[0:64, :], in_=Wog2[:, :])
                nc.vector.tensor_copy(out=wog2_sb, in_=wt4[0:64, :])

                # x -> xT (PE transpose, f32 in -> bf16 out)
                for tt in range(KO):
                    xrow = xr_pool.tile([128, D], F32, tag="xrow")
                    nc.sync.dma_start(out=xrow, in_=x2[:, tt, :])
                    for j in range(KO):
                        pst = pstp.tile([128, 128], F32, tag="pst")
                        nc.tensor.transpose(pst,
                                            xrow[:, j * 128:(j + 1) * 128],
                                            ident32)
                        nc.vector.tensor_copy(
                            out=xT[:, j, tt * 128:(tt + 1) * 128], in_=pst)

                # gamma / f1 / og1 projections (T-layout outputs)
                def proj_small(wap, dout, evac):
                    for tb in range(2):
                        ps = pswide.tile([128, 512], F32, tag="projT")
                        for ko in range(KO):
                            nc.tensor.matmul(
                                ps[:dout, :], wap[:, ko, :],
                                xT[:, ko, tb * 512:(tb + 1) * 512],
                                start=(ko == 0), stop=(ko == KO - 1))
                        evac(ps, tb)

                proj_small(wgam_sb, 16, lambda ps, tb: nc.scalar.activation(
                    out=gamT[:, tb * 512:(tb + 1) * 512], in_=ps[:16, :],
                    func=AF.Sigmoid))
                proj_small(wf1_sb, 64, lambda ps, tb: nc.scalar.copy(
                    out=f1T[:, tb * 512:(tb + 1) * 512], in_=ps[:64, :]))
                proj_small(wog1_sb, 64, lambda ps, tb: nc.scalar.copy(
                    out=og1T[:, tb * 512:(tb + 1) * 512], in_=ps[:64, :]))

                # gam_t (negated) + ginv
                for cch in range(NCH):
                    pst = pstp.tile([128, 128], F32, tag="pst")
                    nc.tensor.transpose(
                        pst[:, 0:16], gamT[:, cch * 128:(cch + 1) * 128],
                        ident32[0:16, 0:16])
                    nc.vector.tensor_scalar_mul(gam_t[:, cch, :],
                                                pst[:, 0:16], -1.0)
                nc.vector.reciprocal_approx_fast(out=ginv_t[:, :, :],
                                                 in_=gam_t[:, :, :])

                # gk: f2 proj -> sigmoid -> ln -> per-64 cumsum -> gT
                # (activations batched per function to limit table reloads)
                for ko in range(KO):
                    sp = sp_pool.tile([128, TT], F32, tag="sp")
                    for tb in range(2):
                        ps = pswide.tile([128, 512], F32, tag="projT")
                        nc.tensor.matmul(ps,
                                         wf2_sb[:, ko * 128:(ko + 1) * 128],
                                         f1T[:, tb * 512:(tb + 1) * 512],
                                         start=True, stop=True)
                        nc.scalar.activation(
                            out=sp[:, tb * 512:(tb + 1) * 512], in_=ps,
                            func=AF.Sigmoid)
                    nc.scalar.activation(out=sp, in_=sp, func=AF.Ln)
                    for b in range(16):
                        nc.vector.tensor_tensor_scan(
                            out=gT[:, ko, b * 64:(b + 1) * 64],
                            data0=sp[:, b * 64:(b + 1) * 64],
                            data1=sp[:, b * 64:(b + 1) * 64],
                            initial=0.0, op0=ALU.add, op1=ALU.bypass)
                    Ee = gT[:, ko, 63::128]     # even-block ends   [128, 8]
                    Eo = gT[:, ko, 127::128]    # odd-block ends
                    Me = gT[:, ko, 31::128]     # even-block mids
                    Mo = gT[:, ko, 95::128]     # odd-block mids
                    nc.vector.tensor_scalar_mul(negmid[:, ko, 0::2], Me, -1.0)
                    nc.vector.tensor_scalar_mul(negmid[:, ko, 1::2], Mo, -1.0)
                    t8 = small.tile([128, 8], F32, tag="t8")
                    nc.vector.tensor_add(out=t8, in0=Ee, in1=Eo)
                    nc.scalar.activation(out=lamEndE[:, ko, :], in_=t8,
                                         func=AF.Exp)
                    nc.vector.tensor_sub(out=t8, in0=Ee, in1=Me)
                    nc.vector.tensor_add(out=t8, in0=t8, in1=Mo)
                    nc.scalar.activation(out=crossM[:, ko, :], in_=t8,
                                         func=AF.Exp)
                    nc.vector.tensor_add(out=t8, in0=Ee, in1=Eo)
                    nc.vector.tensor_sub(out=t8, in0=t8, in1=Me)
                    nc.scalar.activation(out=kshift[:, ko, 0::2], in_=t8,
                                         func=AF.Exp)
                    nc.vector.tensor_sub(out=t8, in0=Eo, in1=Mo)
                    nc.scalar.activation(out=kshift[:, ko, 1::2], in_=t8,
                                         func=AF.Exp)
                    nc.scalar.activation(out=qshift[:, ko, 0::2], in_=Me,
                                         func=AF.Exp)
                    nc.vector.tensor_add(out=t8, in0=Mo, in1=Ee)
                    nc.scalar.activation(out=qshift[:, ko, 1::2], in_=t8,
                                         func=AF.Exp)

                # og2 -> sigmoid -> DRAM
                for ko in range(KO):
                    for tb in range(2):
                        ps = pswide.tile([128, 512], F32, tag="projT")
                        nc.tensor.matmul(ps,
                                         wog2_sb[:, ko * 128:(ko + 1) * 128],
                                         og1T[:, tb * 512:(tb + 1) * 512],
                                         start=True, stop=True)
                        ogt = sp_pool.tile([128, 512], BF16, tag="ogt")
                        nc.scalar.activation(out=ogt, in_=ps, func=AF.Sigmoid)
                        nc.sync.dma_start(
                            out=d_ogT[:, ko, tb * 512:(tb + 1) * 512],
                            in_=ogt)

            def load_wbf(w3ap):
                wslot = wslot_u
                for kk in range(KO):
                    wt = wtmp_pool.tile([128, D], F32, tag="wtmp")
                    nc.sync.dma_start(out=wt, in_=w3ap[:, kk, :])
                    nc.vector.tensor_copy(out=wslot[:, kk, :], in_=wt)
                return wslot

            # v projection (token layout) - x stationary, 2 MMs per LDW
            wslot = load_wbf(wv3)
            for tt in range(NCH):
                psA = pswide.tile([128, 512], F32, tag="projT")
                psB = pswide.tile([128, 512], F32, tag="projT")
                for kk in range(KO):
                    lhs = xT[:, kk, tt * 128:(tt + 1) * 128]
                    nc.tensor.matmul(psA, lhs, wslot[:, kk, 0:512],
                                     start=(kk == 0), stop=(kk == KO - 1))
                    nc.tensor.matmul(psB, lhs, wslot[:, kk, 512:1024],
                                     start=(kk == 0), stop=(kk == KO - 1))
                nc.vector.tensor_copy(out=v_t[:, tt, 0:512], in_=psA)
                nc.vector.tensor_copy(out=v_t[:, tt, 512:1024], in_=psB)

            # q projection -> hatQ
            wslot = load_wbf(wq3)
            for ko in range(KO):
                psA = pswide.tile([128, 512], F32, tag="projT")
                psB = pswide.tile([128, 512], F32, tag="projT")
                for kk in range(KO):
                    lhs = wslot[:, kk, ko * 128:(ko + 1) * 128]
                    nc.tensor.matmul(psA, lhs, xT[:, kk, 0:512],
                                     start=(kk == 0), stop=(kk == KO - 1))
                    nc.tensor.matmul(psB, lhs, xT[:, kk, 512:1024],
                                     start=(kk == 0), stop=(kk == KO - 1))
                qs0 = tmp.tile([128, 512], F32, tag="qs")
                nc.scalar.activation(out=qs0, in_=psA, func=AF.Silu)
                qs1 = tmp.tile([128, 512], F32, tag="qs")
                nc.scalar.activation(out=qs1, in_=psB, func=AF.Silu)
                for tb, qs in ((0, qs0), (1, qs1)):
                    eg = tmp.tile([128, 512], F32, tag="eg")
                    for cc in range(8):
                        b = tb * 8 + cc
                        nc.scalar.activation(
                            out=eg[:, cc * 64:(cc + 1) * 64],
                            in_=gT[:, ko, b * 64:(b + 1) * 64],
                            func=AF.Exp, bias=negmid[:, ko, b:b + 1])
                    nc.vector.tensor_mul(
                        out=hatQ[:, ko, tb * 512:(tb + 1) * 512],
                        in0=qs, in1=eg)

            # k projection -> hatK, hatA
            wslot = load_wbf(wk3)
            for ko in range(KO):
                psA = pswide.tile([128, 512], F32, tag="projT")
                psB = pswide.tile([128, 512], F32, tag="projT")
                for kk in range(KO):
                    lhs = wslot[:, kk, ko * 128:(ko + 1) * 128]
                    nc.tensor.matmul(psA, lhs, xT[:, kk, 0:512],
                                     start=(kk == 0), stop=(kk == KO - 1))
                    nc.tensor.matmul(psB, lhs, xT[:, kk, 512:1024],
                                     start=(kk == 0), stop=(kk == KO - 1))
                ks0 = ktmp_pool.tile([128, 512], F32, tag="ks")
                nc.scalar.activation(out=ks0, in_=psA, func=AF.Silu)
                ks1 = ktmp_pool.tile([128, 512], F32, tag="ks")
                nc.scalar.activation(out=ks1, in_=psB, func=AF.Silu)
                rps = []
                for tb, ks in ((0, ks0), (1, ks1)):
                    k2 = tmp.tile([128, 512], F32R, tag="k2r")
                    nc.vector.tensor_mul(out=k2, in0=ks, in1=ks)
                    pn = pn_pool.tile([2, 512], F32, tag="pn")
                    nc.tensor.matmul(pn, ones_l2, k2, start=True, stop=True)
                    nrm = small.tile([2, 512], F32, tag="nrm")
                    nc.vector.tensor_scalar_max(nrm, pn, 1e-24)
                    rp = rp_pool.tile([2, 512], F32, tag="rp")
                    nc.vector.reciprocal_approx_fast(out=rp, in_=nrm)
                    rps.append(rp)
                kns = []
                for tb, ks in ((0, ks0), (1, ks1)):
                    rinv = small.tile([2, 512], F32R, tag="rinv")
                    nc.scalar.activation(out=rinv, in_=rps[tb], func=AF.Sqrt)
                    bcn = bc_pool.tile([128, 512], F32, tag="bc")
                    nc.tensor.matmul(bcn, ones2T, rinv, start=True,
                                     stop=True)
                    kn = ktmp_pool.tile([128, 512], F32, tag="kn")
                    nc.vector.tensor_mul(out=kn, in0=ks, in1=bcn)
                    kns.append(kn)
                for tb in range(2):
                    kn = kns[tb]
                    # hatK = kn * exp(-(g - mid))
                    egn = tmp.tile([128, 512], F32, tag="eg")
                    for cc in range(8):
                        b = tb * 8 + cc
                        nc.scalar.activation(
                            out=egn[:, cc * 64:(cc + 1) * 64],
                            in_=gT[:, ko, b * 64:(b + 1) * 64],
                            func=AF.Exp, scale=-1.0,
                            bias=gT[:, ko, b * 64 + 31:b * 64 + 32])
                    nc.vector.tensor_mul(
                        out=hatK[:, ko, tb * 512:(tb + 1) * 512],
                        in0=kn, in1=egn)
                    # hatA = kn * exp(2g - gprev - mid)
                    twog = tmp.tile([128, 512], F32, tag="twog")
                    gsl = gT[:, ko, tb * 512:(tb + 1) * 512]
                    nc.vector.tensor_scalar_mul(twog, gsl, 2.0)
                    for cc in range(8):
                        sl = slice(cc * 64 + 1, (cc + 1) * 64)
                        slp = slice(cc * 64, (cc + 1) * 64 - 1)
                        nc.vector.tensor_sub(out=twog[:, sl], in0=twog[:, sl],
                                             in1=gsl[:, slp])
                    for cc in range(8):
                        b = tb * 8 + cc
                        nc.scalar.activation(
                            out=twog[:, cc * 64:(cc + 1) * 64],
                            in_=twog[:, cc * 64:(cc + 1) * 64],
                            func=AF.Exp, bias=negmid[:, ko, b:b + 1])
                    nc.vector.tensor_mul(
                        out=hatA[:, ko, tb * 512:(tb + 1) * 512],
                        in0=twog, in1=kn)

            if DBG:
                nc.sync.dma_start(out=g("dbg_gT")[:, :, :], in_=gT)
                nc.sync.dma_start(out=g("dbg_hatK")[:, :, :], in_=hatK)
                nc.sync.dma_start(out=g("dbg_hatA")[:, :, :], in_=hatA)
                nc.sync.dma_start(out=g("dbg_hatQ")[:, :, :], in_=hatQ)

        # ============ P2: phase A per (chunk, head) ============
        # P2/P3 products alias dead P1 tensors (same byte size per tile)
        pnd_bd = xTu.rearrange("p ko (nc q) -> p ko nc q", q=128)
        gt_all = wslot_u.rearrange("p ko (nc q) -> p ko nc q", q=C)
        oT = gTu
        nc.vector.memset(pnd_bd, 0.0)

        with tc.tile_pool(name="vab", bufs=3) as va_pool, \
             tc.tile_pool(name="kbb", bufs=3) as kb_pool, \
             tc.tile_pool(name="sc", bufs=3) as sc_pool, \
             tc.tile_pool(name="wm", bufs=4) as wm_pool, \
             tc.tile_pool(name="zp", bufs=8) as z_pool, \
             tc.tile_pool(name="psg", bufs=2, space="PSUM") as psg_pool, \
             tc.tile_pool(name="psx", bufs=2, space="PSUM") as psx_pool, \
             tc.tile_pool(name="psf", bufs=2, space="PSUM") as psf_pool, \
             tc.tile_pool(name="pstr", bufs=2, space="PSUM") as pstr_pool:
            for c in range(NCH):
                csl = slice(c * C, (c + 1) * C)
                s1 = slice(c * C, c * C + 64)
                s2 = slice(c * C + 64, (c + 1) * C)
                va_buf = va_pool.tile([128, H, C], BF16, tag="va")
                kb_buf = kb_pool.tile([128, H, HD], BF16, tag="kb")
                k1c_all = sc_pool.tile([128, KO, 64], BF16, tag="k1c")
                for ko in range(KO):
                    # scaled K1 for the b1->b2 coupling
                    nc.vector.tensor_scalar_mul(
                        k1c_all[:, ko, :], hatK[:, ko, s1],
                        crossM[:, ko, c:c + 1])
                    # true A^T columns for the va rhs
                    atr = sc_pool.tile([128, 128], BF16, tag="atr")
                    nc.vector.tensor_scalar_mul(
                        atr[:, 0:64], hatA[:, ko, s1],
                        qshift[:, ko, 2 * c:2 * c + 1])
                    nc.vector.tensor_scalar_mul(
                        atr[:, 64:128], hatA[:, ko, s2],
                        qshift[:, ko, 2 * c + 1:2 * c + 2])
                    psT = pstr_pool.tile([128, 128], BF16, tag="psT")
                    nc.tensor.transpose(psT, atr, identB)
                    nc.scalar.copy(out=va_buf[:, 2 * ko, 64:128],
                                   in_=psT[:, 0:64])
                    nc.scalar.copy(out=va_buf[:, 2 * ko + 1, 64:128],
                                   in_=psT[:, 64:128])
                    # va_V = v / gamma
                    for par in range(2):
                        h = 2 * ko + par
                        nc.vector.tensor_scalar_mul(
                            va_buf[:, h, 0:64],
                            v_t[:, c, h * 64:(h + 1) * 64],
                            ginv_t[:, c, h:h + 1])
                    # kbar = hatK * kshift -> transpose -> gamma scale
                    kbt = sc_pool.tile([128, 128], BF16, tag="kbt")
                    nc.vector.tensor_scalar_mul(
                        kbt[:, 0:64], hatK[:, ko, s1],
                        kshift[:, ko, 2 * c:2 * c + 1])
                    nc.vector.tensor_scalar_mul(
                        kbt[:, 64:128], hatK[:, ko, s2],
                        kshift[:, ko, 2 * c + 1:2 * c + 2])
                    psK = pstr_pool.tile([128, 128], BF16, tag="psT")
                    nc.tensor.transpose(psK, kbt, identB)
                    nc.vector.tensor_scalar_mul(
                        kb_buf[:, 2 * ko, :], psK[:, 0:64],
                        gam_t[:, c, 2 * ko:2 * ko + 1])
                    nc.vector.tensor_scalar_mul(
                        kb_buf[:, 2 * ko + 1, :], psK[:, 64:128],
                        gam_t[:, c, 2 * ko + 1:2 * ko + 2])
                if DBG:
                    nc.sync.dma_start(out=g("dbg_va")[:, c, :, :], in_=va_buf)
                    nc.sync.dma_start(out=g("dbg_kb")[:, c, :, :], in_=kb_buf)

                for h in range(H):
                    hb = (h % 2) * 64
                    ko = h // 2
                    hsl = slice(hb, hb + 64)
                    kS1 = hatK[hsl, ko, s1]
                    kS2 = hatK[hsl, ko, s2]
                    k1c = k1c_all[hsl, ko, :]
                    aS1 = hatA[hsl, ko, s1]
                    aS2 = hatA[hsl, ko, s2]
                    qS1 = hatQ[hsl, ko, s1]
                    qS2 = hatQ[hsl, ko, s2]
                    psg = psg_pool.tile([128, 256], F32, tag="psg")
                    nc.tensor.matmul(psg[0:64, 0:64], kS1, aS1,
                                     start=True, stop=True,
                                     tile_position=(hb, 0))
                    nc.tensor.matmul(psg[0:64, 128:192], kS1, qS1,
                                     start=True, stop=True,
                                     tile_position=(hb, 0))
                    nc.tensor.matmul(psg[0:64, 64:128], k1c, aS2,
                                     start=True, stop=True,
                                     tile_position=(hb, 0))
                    nc.tensor.matmul(psg[0:64, 192:256], k1c, qS2,
                                     start=True, stop=True,
                                     tile_position=(hb, 0))
                    nc.tensor.matmul(psg[64:128, 64:128], kS2, aS2,
                                     start=True, stop=True,
                                     tile_position=(hb, 64))
                    nc.tensor.matmul(psg[64:128, 192:256], kS2, qS2,
                                     start=True, stop=True,
                                     tile_position=(hb, 64))
                    # fillers: finite garbage under the mask
                    nc.tensor.matmul(psg[64:128, 0:64], kS2, aS1,
                                     start=True, stop=True,
                                     tile_position=(hb, 64))
                    nc.tensor.matmul(psg[64:128, 128:192], kS2, qS1,
                                     start=True, stop=True,
                                     tile_position=(hb, 64))
                    wm = wm_pool.tile([128, 256], BF16, tag="wm")
                    nc.vector.scalar_tensor_tensor(
                        out=wm, in0=psg, scalar=gam_t[:, c, h:h + 1],
                        in1=maskWM, op0=ALU.mult, op1=ALU.mult)
                    va = va_buf[:, h, :]
                    zc = va
                    for it in range(NEU):
                        psx = psx_pool.tile([128, 128], F32, tag="psx")
                        nc.tensor.matmul(psx, wm[:, 0:128], zc,
                                         start=True, stop=True)
                        zn = z_pool.tile([128, 128], BF16, tag="zn")
                        nc.vector.tensor_add(out=zn, in0=psx, in1=va)
                        zc = zn
                    kb = kb_buf[:, h, :]
                    psf = psf_pool.tile([128, 384], F32, tag="psf")
                    nc.tensor.matmul(psf[hsl, 0:64], zc[:, 64:128], kb,
                                     start=True, stop=True,
                                     tile_position=(0, hb))
                    nc.tensor.matmul(psf[hsl, 64:192], zc[:, 64:128],
                                     wm[:, 128:256],
                                     start=True, stop=True,
                                     tile_position=(0, hb))
                    nc.tensor.matmul(psf[hsl, 192:320], zc[:, 0:64],
                                     wm[:, 128:256],
                                     start=True, stop=True,
                                     tile_position=(0, hb))
                    nc.tensor.matmul(psf[hsl, 320:384], kb, zc[:, 0:64],
                                     start=True, stop=True,
                                     tile_position=(0, hb))
                    nc.scalar.copy(out=pnd_bd[hsl, ko, c, hb:hb + 64],
                                   in_=psf[hsl, 0:64])
                    nc.scalar.copy(out=gt_all[hsl, ko, c, :],
                                   in_=psf[hsl, 64:192])
                    nc.vector.tensor_copy(out=oT[hsl, ko, csl],
                                          in_=psf[hsl, 192:320])
                    nc.vector.tensor_copy(out=ds_all[hsl, ko, c, :],
                                          in_=psf[hsl, 320:384])

        if DBG:
            nc.sync.dma_start(out=g("dbg_pnd")[:, :, :, :], in_=pnd_bd)
            nc.sync.dma_start(out=g("dbg_gt")[:, :, :, :], in_=gt_all)
            nc.sync.dma_start(out=g("dbg_ds")[:, :, :, :], in_=ds_all)

        # ============ P3: scans, output, layernorm, Wo ============
        with tc.tile_pool(name="p3", bufs=1) as p3, \
             tc.tile_pool(name="q3", bufs=3) as q3_pool, \
             tc.tile_pool(name="tmp3", bufs=2) as tmp3, \
             tc.tile_pool(name="sb3", bufs=2) as sb3_pool, \
             tc.tile_pool(name="psS", bufs=2, space="PSUM") as psS_pool, \
             tc.tile_pool(name="psO", bufs=2, space="PSUM") as psO_pool, \
             tc.tile_pool(name="ln3", bufs=2, space="PSUM") as ppool3, \
             tc.tile_pool(name="ps3", bufs=2, space="PSUM") as pswide3:
            scur = p3.tile([128, KO, HD], F32, tag="scur")
            nc.vector.memset(scur[:, :, :], 0.0)

            def scan_update(c, sbf):
                psS = psS_pool.tile([128, 512], F32, tag="psS")
                for ko in range(KO):
                    nc.tensor.matmul(psS[:, ko * 64:(ko + 1) * 64],
                                     pnd_bd[:, ko, c, :], sbf[:, ko, :],
                                     start=True, stop=True)
                for ko in range(KO):
                    nc.vector.scalar_tensor_tensor(
                        out=scur[:, ko, :], in0=scur[:, ko, :],
                        scalar=lamEndE[:, ko, c:c + 1],
                        in1=psS[:, ko * 64:(ko + 1) * 64],
                        op0=ALU.mult, op1=ALU.add)
                nc.vector.tensor_add(out=scur[:, :, :], in0=scur[:, :, :],
                                     in1=ds_all[:, :, c, :])

            # pass 1: local final state
            for c in range(NCH):
                sbf = sb3_pool.tile([128, KO, HD], BF16, tag="sbf")
                nc.vector.tensor_copy(out=sbf[:, :, :], in_=scur[:, :, :])
                scan_update(c, sbf)

            # AllGather pair exchange + parity mask
            cin3 = cc_in.rearrange("(ko p) f -> p ko f", p=128)
            cout3 = cc_out.rearrange("(r ko p) f -> r p ko f", p=128, r=2)
            nc.sync.dma_start(out=cin3, in_=scur)
            nc.gpsimd.collective_compute(
                "AllGather", ALU.bypass,
                replica_groups=[[0, 1], [2, 3], [4, 5], [6, 7]],
                ins=[cc_in[:, :]], outs=[cc_out[:, :]])
            sinit = p3.tile([128, KO, HD], F32, tag="sinit")
            nc.sync.dma_start(out=sinit, in_=cout3[0])
            par_col = p3.tile([128, 1], F32, tag="par_col")
            nc.sync.dma_start(out=par_col,
                              in_=parity[0:1, 0:1].to_broadcast((128, 1)))
            nc.vector.tensor_scalar_mul(scur[:, :, :], sinit[:, :, :],
                                        par_col)

            # pass 2: outputs + scan
            sbd_z = p3.tile([128, KO, 128], BF16, tag="sbd")
            nc.vector.memset(sbd_z[:, :, :], 0.0)
            for c in range(NCH):
                csl = slice(c * C, (c + 1) * C)
                sbf = sb3_pool.tile([128, KO, HD], BF16, tag="sbf")
                nc.vector.tensor_copy(out=sbf[:, :, :], in_=scur[:, :, :])
                # block-diag copy for the output matmuls
                nc.vector.tensor_scalar_mul(sbd_z[0:64, :, 0:64],
                                            sbf[0:64, :, :], 1.0)
                nc.vector.tensor_scalar_mul(sbd_z[64:128, :, 64:128],
                                            sbf[64:128, :, :], 1.0)
                for ko in range(KO):
                    # true Qglob^T slice: hatQ scaled by qshift per 64-block
                    qtm = q3_pool.tile([128, 128], BF16, tag="qtm")
                    nc.vector.tensor_scalar_mul(
                        qtm[:, 0:64], hatQ[:, ko, c * C:c * C + 64],
                        qshift[:, ko, 2 * c:2 * c + 1])
                    nc.vector.tensor_scalar_mul(
                        qtm[:, 64:128], hatQ[:, ko, c * C + 64:(c + 1) * C],
                        qshift[:, ko, 2 * c + 1:2 * c + 2])
                    psO = psO_pool.tile([128, 128], F32, tag="psO")
                    nc.tensor.matmul(psO, sbd_z[:, ko, :],
                                     gt_all[:, ko, c, :],
                                     start=True, stop=False)
                    nc.tensor.matmul(psO, sbd_z[:, ko, :], qtm,
                                     start=False, stop=True)
                    nc.vector.tensor_add(out=oT[:, ko, csl],
                                         in0=oT[:, ko, csl], in1=psO)
                if c < NCH - 1:
                    scan_update(c, sbf)

            # output gate
            for ko in range(KO):
                for tb in range(2):
                    ogt = tmp3.tile([128, 512], BF16, tag="ogt3")
                    nc.sync.dma_start(
                        out=ogt, in_=d_ogT[:, ko, tb * 512:(tb + 1) * 512])
                    nc.vector.tensor_mul(
                        out=oT[:, ko, tb * 512:(tb + 1) * 512],
                        in0=oT[:, ko, tb * 512:(tb + 1) * 512], in1=ogt)

            # layernorm stats (feature dim = partitions x ko)
            oTb = hatK   # dead after P2; reused as normalized-output buffer
            stat_mu = p3.tile([2, TT], F32R, tag="stat_mu")
            stat_rs = p3.tile([2, TT], F32R, tag="stat_rs")
            for tb in range(2):
                tsl = slice(tb * 512, (tb + 1) * 512)
                psm = ppool3.tile([2, 512], F32, tag="acc")
                for ko in range(KO):
                    nc.tensor.matmul(psm, onesF, oT[:, ko, tsl],
                                     start=(ko == 0), stop=(ko == KO - 1))
                # both rows hold the full-D sum; fold the K=2 bcast double
                nc.vector.tensor_scalar_mul(stat_mu[:, tsl], psm, 0.5 / D)
                ps2 = ppool3.tile([2, 512], F32, tag="acc")
                for ko in range(KO):
                    o2 = tmp3.tile([128, 512], F32R, tag="o2")
                    nc.vector.tensor_mul(out=o2, in0=oT[:, ko, tsl],
                                         in1=oT[:, ko, tsl])
                    nc.tensor.matmul(ps2, onesF, o2,
                                     start=(ko == 0), stop=(ko == KO - 1))
                msq = tmp3.tile([2, 512], F32, tag="msq")
                nc.vector.tensor_scalar_mul(msq, ps2, 1.0 / D)
                mu2 = tmp3.tile([2, 512], F32, tag="mu2")
                nc.vector.tensor_mul(out=mu2, in0=stat_mu[:, tsl],
                                     in1=stat_mu[:, tsl])
                var = tmp3.tile([2, 512], F32, tag="var")
                nc.vector.scalar_tensor_tensor(
                    out=var, in0=mu2, scalar=-4.0, in1=msq,
                    op0=ALU.mult, op1=ALU.add)
                nc.scalar.activation(out=var, in_=var, func=AF.Sqrt,
                                     bias=eps_sb)
                rs = tmp3.tile([2, 512], F32, tag="rs")
                nc.vector.reciprocal_approx_fast(out=rs, in_=var)
                nc.vector.tensor_scalar_mul(stat_rs[:, tsl], rs, 0.5)
            for tb in range(2):
                tsl = slice(tb * 512, (tb + 1) * 512)
                bmu = pswide3.tile([128, 512], F32, tag="projT")
                nc.tensor.matmul(bmu, ones2F, stat_mu[:, tsl],
                                 start=True, stop=True)
                brs = pswide3.tile([128, 512], F32, tag="projT")
                nc.tensor.matmul(brs, ones2F, stat_rs[:, tsl],
                                 start=True, stop=True)
                for ko in range(KO):
                    osl = oT[:, ko, tsl]
                    t1 = tmp3.tile([128, 512], F32, tag="t1f")
                    nc.vector.tensor_sub(out=t1, in0=osl, in1=bmu)
                    t2 = tmp3.tile([128, 512], F32, tag="t2f")
                    nc.vector.tensor_mul(out=t2, in0=t1, in1=brs)
                    nc.vector.tensor_scalar_mul(
                        oTb[:, ko, tsl], t2, nw_sb[:, ko:ko + 1])

            if DBG:
                nc.sync.dma_start(out=g("dbg_oT")[:, :, :], in_=oT)

            # final Wo (staged into hatA, dead after P2)
            wslot = hatA
            for kk in range(KO):
                wt = wtmp_u.tile([128, D], F32, tag="wtmp")
                nc.sync.dma_start(out=wt, in_=wo3[:, kk, :])
                nc.vector.tensor_copy(out=wslot[:, kk, :], in_=wt)
            y2 = ys.rearrange("(tt p) d -> p tt d", p=128)
            for tt in range(NCH):
                psA = pswide3.tile([128, 512], F32, tag="projT")
                psB = pswide3.tile([128, 512], F32, tag="projT")
                for kk in range(KO):
                    lhs = oTb[:, kk, tt * 128:(tt + 1) * 128]
                    nc.tensor.matmul(psA, lhs, wslot[:, kk, 0:512],
                                     start=(kk == 0), stop=(kk == KO - 1))
                    nc.tensor.matmul(psB, lhs, wslot[:, kk, 512:1024],
                                     start=(kk == 0), stop=(kk == KO - 1))
                yrow = tmp3.tile([128, D], F32, tag="yrow")
                nc.vector.tensor_copy(out=yrow[:, 0:512], in_=psA)
                nc.vector.tensor_copy(out=yrow[:, 512:1024], in_=psB)
                nc.sync.dma_start(out=y2[:, tt, :], in_=yrow)


_NC = None


def _get_nc():
    global _NC
    if _NC is None:
        _NC = build()
    return _NC


def kernel(**inputs):
    nc = _get_nc()
    x = np.ascontiguousarray(np.asarray(inputs["x"], dtype=np.float32))
    names = ["Wq", "Wk", "Wv", "Wgamma", "Wf1", "Wf2", "Wog1", "Wog2",
             "norm_w", "Wo"]
    w = {n: np.ascontiguousarray(np.asarray(inputs[n], np.float32))
         for n in names}
    in_maps = []
    for core in range(8):
        b, half = core // 2, core % 2
        m = dict(w)
        m["xs"] = np.ascontiguousarray(x[b, half * TT:(half + 1) * TT, :])
        m["parity"] = np.array([[float(half)]], np.float32)
        in_maps.append(m)
    res = run_bass_kernel_spmd(nc, in_maps, core_ids=list(range(8)))
    out = np.empty((B, T, D), np.float32)
    for core in range(8):
        b, half = core // 2, core % 2
        out[b, half * TT:(half + 1) * TT, :] = res.results[core]["ys"]
    return out
